# revision 23
# baseline (speedup 1.0000x reference)
"""Trainium2 Bass kernel for nn_Encoder_44238163149250.

Sharding: data-parallel over batch B=8 -> one batch element per NeuronCore.
Each core computes all five encoder outputs for its batch element.

Per-core structure (batch element b), dist and posenc chunks interleaved in
one Tile region so every engine stays loaded across the whole kernel:

  dist branch  (65536 pts, 512-pt chunks):
      y = relu(w1*log1p(d) + b1) @ W2.T + b2
      - log1p once on ACT over the whole [128,512] input; round-trip through
        a DRAM scratch so chunks re-load as [1,512] rows at base partition 0
        (SWDGE casts f32->fp16 during that reload)
      - broadcast across partitions with a K=1 ones outer-product matmul
      - h1 = Relu(scale*psum + bias) on ACT (per-partition scale/bias), fp16
      - layer 2 uses h1 slices as the matmul *stationary* operand
        (lhsT = h1[:,128j], rhs = W2.T fp16) so PSUM comes out in
        [points, feat] layout -> no transpose anywhere
      - bias-add + PSUM->SBUF on DVE; two chunks batched per output DMA
  posenc branches (frontier+ghist = fparams, phist+agent = aparams):
      - freq-scaled args via one K=3 fp16 matmul vs [idx_r; idx_c; 1]
        (integer idx and 2^k/512 coefficients are exact in fp16)
      - range-reduce to [-pi,pi]: k = round(x/2pi) via ACT Identity
        (scale+magic bias) + DVE subtract; r = x - k*fl32(2pi) via DVE
        scalar_tensor_tensor; clamp on DVE; cos rows are sin(x + pi/2)
        with the shift riding the ones row of the idx matmul
      - 3-layer fp16 MLP, layer biases folded into ACT per-partition bias,
        last layer via the same stationary-operand trick

  Point order inside each 512-pt chunk is host-permuted (position j*128+p
  holds point 4p+j) so each partition's DMA store is one contiguous 2KB run
  instead of 4 scattered 512B runs.
"""

import os

import numpy as np

import concourse.bass as bass
import concourse.bass_utils as _bass_utils
import concourse.mybir as mybir
import concourse.tile as tile
from concourse.bass_utils import run_bass_kernel_spmd

# The repo's walrus invocation passes --enable-ldw-opt=false, which leaves
# every matmul paying its LDWEIGHTS serially (~123ns each, ~100us across this
# kernel). Flip it on for our NEFF compiles.
_orig_run_command = _bass_utils.run_command


def _run_command_ldwopt(argv, **kwargs):
    argv = [
        a
        for a in argv
    ]
    return _orig_run_command(argv, **kwargs)


_bass_utils.run_command = _run_command_ldwopt

F32 = mybir.dt.float32
F16 = mybir.dt.float16
AF = mybir.ActivationFunctionType
ALU = mybir.AluOpType

NUM_FREQS = 10
B = 8

# per-core sizes
ND = 65536          # dist points (D*NF = 8*8192)
NFR = 8192          # frontier pts
NG = 2048           # ghist pts
NPH = 2048          # phist pts
NA = 8              # agent pts
NFB = NFR + NG      # F-branch pts  (fparams + type_frontier)
NAB = 2560          # A-branch pts padded (phist 2048 + agent 8 -> 2560)

PC = 512            # chunk size (points) for both branches
NDC = ND // PC      # 128 dist chunks
NFC = NFB // PC     # 20
NAC = NAB // PC     # 5

MAGIC = 12582912.0                       # 1.5 * 2**23: round-to-nearest trick
TWO_PI = 2.0 * np.pi
FL2PI = float(np.float32(TWO_PI))        # r = x - k*fl32(2pi): |err| <= 5e-5 rad
INV_2PI = 1.0 / TWO_PI
PI_LO = float(np.nextafter(np.float32(np.pi), np.float32(0)))  # f32 just below pi

_PROG = None        # cached nc build
_LEGALIZED = False  # wait-legalization applied to _PROG (HW path only)
LAST_RESULT = None  # BassKernelResults of the last kernel() call (for test.py)

DIST_ON = os.environ.get("K_DIST", "1") == "1"
POSENC_ON = os.environ.get("K_POSENC", "1") == "1"


def _legalize_waits(nc):
    """This toolchain's walrus accepts at most ONE sync wait per instruction
    (codegen raises 'Too many sync wait commands' otherwise), while Tile
    freely emits several. Hoist all but the last wait of each instruction
    onto same-engine EventSemaphore instructions inserted right before it —
    engines execute their queue in order, so sequential waits are equivalent
    to one multi-wait."""
    n = 0
    for f in nc.m.functions:
        for b in f.blocks:
            insts = list(b.instructions)
            out = []
            changed = False
            for i in insts:
                si = i.sync_info
                if si is not None and len(si.on_wait) > 1:
                    waits = list(si.on_wait)
                    for j, w in enumerate(waits[:-1]):
                        ev = mybir.InstEventSemaphore(
                            name=f"lw_{i.name}_{j}",
                            engine=i.engine,
                            ins=[],
                            outs=[],
                            sync_info=mybir.SyncInfo(on_wait=[w], on_update=[]),
                        )
                        out.append(ev)
                        n += 1
                    i.sync_info = mybir.SyncInfo(
                        on_wait=[waits[-1]], on_update=list(si.on_update)
                    )
                    changed = True
                out.append(i)
            if changed:
                cur = b.instructions
                del cur[:]
                for i in out:
                    cur.append(i)
    return n


def _build_program():
    nc = bass.Bass()

    # ---- DRAM I/O ----
    din = nc.dram_tensor("dist", [128, 512], F32, kind="ExternalInput")
    idx_f = nc.dram_tensor("idxf", [3, NFB], F16, kind="ExternalInput")
    idx_a = nc.dram_tensor("idxa", [3, NAB], F16, kind="ExternalInput")
    mmat = nc.dram_tensor("mmat", [3, 40], F16, kind="ExternalInput")
    w1d = nc.dram_tensor("w1d", [128, 1], F32, kind="ExternalInput")
    b1d = nc.dram_tensor("b1d", [128, 1], F32, kind="ExternalInput")
    w2td = nc.dram_tensor("w2td", [128, 128], F16, kind="ExternalInput")
    b2bd = nc.dram_tensor("b2bd", [128, PC], F32, kind="ExternalInput")
    brt = {}
    for s in ("f", "a"):
        brt[s] = dict(
            w1a=nc.dram_tensor(f"w1a{s}", [40, 64], F16, kind="ExternalInput"),
            w1b=nc.dram_tensor(f"w1b{s}", [2, 64], F16, kind="ExternalInput"),
            b1e=nc.dram_tensor(f"b1e{s}", [64, 1], F32, kind="ExternalInput"),
            w2t=nc.dram_tensor(f"w2t{s}", [64, 128], F16, kind="ExternalInput"),
            b2c=nc.dram_tensor(f"b2c{s}", [128, 1], F32, kind="ExternalInput"),
            w3t=nc.dram_tensor(f"w3t{s}", [128, 128], F16, kind="ExternalInput"),
            b3b=nc.dram_tensor(f"b3b{s}", [128, PC], F32, kind="ExternalInput"),
        )

    dout = nc.dram_tensor("dist_out", [ND, 128], F32, kind="ExternalOutput")
    fout = nc.dram_tensor("front_out", [NFR, 128], F32, kind="ExternalOutput")
    gout = nc.dram_tensor("ghist_out", [NG, 128], F32, kind="ExternalOutput")
    pout = nc.dram_tensor("phist_out", [NPH, 128], F32, kind="ExternalOutput")
    aout = nc.dram_tensor("agent_out", [NA, 128], F32, kind="ExternalOutput")

    # Views matching the in-chunk permutation: position j*128+p <-> point
    # 512c + 4p + j. Dist pairs two chunks per DMA.
    dview = dout.rearrange("(t h p g) o -> t p h g o", h=2, p=128, g=4)
    fview = fout.rearrange("(c p g) o -> c p g o", p=128, g=4)
    gview = gout.rearrange("(c p g) o -> c p g o", p=128, g=4)
    pview = pout.rearrange("(c p g) o -> c p g o", p=128, g=4)
    aview = aout.rearrange("(p j) o -> p j o", j=4)

    with tile.TileContext(nc) as tc:
        with tc.tile_pool(name="const", bufs=1) as cp:

            def cload(dram, shape, tag, dt=F32):
                t = cp.tile(shape, dt, tag=tag)
                nc.sync.dma_start(out=t[:], in_=dram[:, :])
                return t

            w1d_t = cload(w1d, [128, 1], "w1d")
            b1d_t = cload(b1d, [128, 1], "b1d")
            w2td_t = cload(w2td, [128, 128], "w2td", F16)
            b2bd_t = cload(b2bd, [128, PC], "b2bd")
            mmat_t = cload(mmat, [3, 40], "mmat", F16)
            brc = {}
            for s in ("f", "a"):
                d = brt[s]
                brc[s] = dict(
                    w1a=cload(d["w1a"], [40, 64], f"w1a{s}", F16),
                    w1b=cload(d["w1b"], [2, 64], f"w1b{s}", F16),
                    b1e=cload(d["b1e"], [64, 1], f"b1e{s}"),
                    w2t=cload(d["w2t"], [64, 128], f"w2t{s}", F16),
                    b2c=cload(d["b2c"], [128, 1], f"b2c{s}"),
                    w3t=cload(d["w3t"], [128, 128], f"w3t{s}", F16),
                    b3b=cload(d["b3b"], [128, PC], f"b3b{s}"),
                )
            idxf_t = cload(idx_f, [3, NFB], "idxf", F16)
            idxa_t = cload(idx_a, [3, NAB], "idxa", F16)

            ones_l = cp.tile([1, 128], F16, tag="ones")
            nc.vector.memset(ones_l[:], 1.0)
            magic_c = cp.tile([40, 1], F32, tag="magic")
            nc.vector.memset(magic_c[:], MAGIC)

            # HAM warmup: the PE clock-gate defaults to K=4/8 (1.2 GHz) and
            # only reaches 2.4 GHz after ~3.4us of *continuous* matmul
            # activity. This kernel's real matmul bursts are ~0.9us, so
            # without a warmup the whole kernel streams at half clock
            # (measured: zero HAM transitions, N=128 MMs at 107ns spacing).
            # Burn ~5us of back-to-back K=1 matmuls on a scratch bank first;
            # afterwards no PE gap exceeds the ~3.4us re-throttle window.
            warm_r = cp.tile([1, 512], F16, tag="warm_r")
            nc.vector.memset(warm_r[:], 1.0)
            with tc.tile_pool(name="warmp", bufs=1, space="PSUM") as warmp:
                wps = warmp.tile([128, 512], F32, tag="wps")
                for _ in range(14):
                    nc.tensor.matmul(
                        wps[:], lhsT=ones_l[:], rhs=warm_r[:],
                        start=True, stop=True,
                    )

            # dist input + log1p, once (input already host-permuted)
            dist_sb = cp.tile([128, 512], F32, tag="dist_sb")
            nc.sync.dma_start(out=dist_sb[:], in_=din[:, :])
            logd = cp.tile([128, 512], F32, tag="logd")
            nc.scalar.activation(logd[:], dist_sb[:], AF.Ln, bias=1.0, scale=1.0)

            with (
                tc.tile_pool(name="ddram", bufs=1, space="DRAM") as ddram,
                tc.tile_pool(name="dsb", bufs=4) as dsb,
                tc.tile_pool(name="esb", bufs=4) as esb,
                tc.tile_pool(name="ppA", bufs=4, space="PSUM") as ppA,
                tc.tile_pool(name="ppB", bufs=4, space="PSUM") as ppB,
            ):
                scr = ddram.tile([128, 512], F32, tag="scr")
                nc.gpsimd.dma_start(out=scr[:], in_=logd[:])

                def dist_chunk(c, ot_half):
                    ct = dsb.tile([1, PC], F16, tag="ct")
                    nc.gpsimd.dma_start(out=ct[:], in_=scr[c : c + 1, :])
                    psb = ppA.tile([128, PC], F32, tag="psA")
                    nc.tensor.matmul(
                        psb[:], lhsT=ones_l[:], rhs=ct[0:1, :],
                        start=True, stop=True,
                    )
                    h1 = dsb.tile([128, PC], F16, tag="h1")
                    nc.scalar.activation(
                        h1[:], psb[:], AF.Relu, bias=b1d_t[:], scale=w1d_t[:]
                    )
                    pso = ppB.tile([128, PC], F32, tag="psB")
                    for j in range(PC // 128):
                        nc.tensor.matmul(
                            pso[:, j * 128 : (j + 1) * 128],
                            lhsT=h1[:, j * 128 : (j + 1) * 128],
                            rhs=w2td_t[:],
                            start=True,
                            stop=True,
                        )
                    nc.vector.tensor_add(ot_half, pso[:], b2bd_t[:])

                def posenc_chunk(s, idx_t, c):
                    w = brc[s]
                    lo = c * PC
                    pss_full = ppA.tile([128, PC], F32, tag="psA", name="pss_full")
                    pss = pss_full[0:40, :]
                    nc.tensor.matmul(
                        pss[:], lhsT=mmat_t[:], rhs=idx_t[:, lo : lo + PC],
                        start=True, stop=True,
                    )
                    # k = round(args/2pi): scale+magic on ACT, subtract on DVE
                    kt1 = esb.tile([40, PC], F32, tag="kt1")
                    nc.scalar.activation(
                        kt1[:], pss[:], AF.Identity,
                        bias=magic_c[:], scale=INV_2PI,
                    )
                    kt = esb.tile([40, PC], F32, tag="kt")
                    nc.vector.tensor_scalar(
                        out=kt[:], in0=kt1[:], scalar1=MAGIC, scalar2=None,
                        op0=ALU.subtract,
                    )
                    rt = esb.tile([40, PC], F32, tag="rt")
                    nc.vector.scalar_tensor_tensor(
                        out=rt[:], in0=kt[:], scalar=-FL2PI,
                        in1=pss[:], op0=ALU.mult, op1=ALU.add,
                    )
                    r3 = esb.tile([40, PC], F32, tag="r3")
                    nc.vector.tensor_scalar(
                        out=r3[:], in0=rt[:],
                        scalar1=PI_LO, scalar2=-PI_LO,
                        op0=ALU.min, op1=ALU.max,
                    )
                    xs = esb.tile([40, PC], F16, tag="xs")
                    nc.scalar.activation(xs[:], r3[:], AF.Sin)

                    ps1_full = ppA.tile([128, PC], F32, tag="psA", name="ps1_full")
                    ps1 = ps1_full[0:64, :]
                    nc.tensor.matmul(
                        ps1[:], lhsT=w["w1a"][:], rhs=xs[:],
                        start=True, stop=False,
                    )
                    nc.tensor.matmul(
                        ps1[:], lhsT=w["w1b"][:], rhs=idx_t[0:2, lo : lo + PC],
                        start=False, stop=True,
                    )
                    h1e = esb.tile([64, PC], F16, tag="h1e")
                    nc.scalar.activation(h1e[:], ps1[:], AF.Relu, bias=w["b1e"][:])

                    ps2 = ppA.tile([128, PC], F32, tag="psA")
                    nc.tensor.matmul(
                        ps2[:], lhsT=w["w2t"][:], rhs=h1e[:], start=True, stop=True
                    )
                    h2e = esb.tile([128, PC], F16, tag="h2e")
                    nc.scalar.activation(h2e[:], ps2[:], AF.Relu, bias=w["b2c"][:])

                    po = ppB.tile([128, PC], F32, tag="psB")
                    for j in range(PC // 128):
                        nc.tensor.matmul(
                            po[:, j * 128 : (j + 1) * 128],
                            lhsT=h2e[:, j * 128 : (j + 1) * 128],
                            rhs=w["w3t"][:],
                            start=True,
                            stop=True,
                        )
                    oute = esb.tile([128, PC], F32, tag="oute")
                    nc.vector.tensor_add(oute[:], po[:], w["b3b"][:])

                    src = oute[:].rearrange("p (g o) -> p g o", g=4)
                    if s == "f":
                        if c < NFR // PC:
                            nc.sync.dma_start(out=fview[c, :, :, :], in_=src)
                        else:
                            nc.sync.dma_start(
                                out=gview[c - NFR // PC, :, :, :], in_=src
                            )
                    else:
                        if c < NPH // PC:
                            nc.sync.dma_start(out=pview[c, :, :, :], in_=src)
                        else:
                            nc.sync.dma_start(
                                out=aview[:, :, :],
                                in_=oute[0:2, :].rearrange("p (j o) -> p j o", j=4),
                            )

                pe_jobs = []
                if POSENC_ON:
                    pe_jobs = [("f", idxf_t, c) for c in range(NFC)] + [
                        ("a", idxa_t, c) for c in range(NAC)
                    ]
                n_pairs = NDC // 2 if DIST_ON else 0
                emitted = 0
                for t in range(n_pairs):
                    ot = dsb.tile([128, 2 * PC], F32, tag="ot")
                    for h in range(2):
                        dist_chunk(2 * t + h, ot[:, h * PC : (h + 1) * PC])
                    nc.sync.dma_start(
                        out=dview[t, :, :, :, :],
                        in_=ot[:].rearrange("p (h g o) -> p h g o", h=2, g=4),
                    )
                    want = (t + 1) * len(pe_jobs) // n_pairs
                    while emitted < want:
                        posenc_chunk(*pe_jobs[emitted])
                        emitted += 1
                while emitted < len(pe_jobs):
                    posenc_chunk(*pe_jobs[emitted])
                    emitted += 1
    return nc


def _get_program():
    global _PROG
    if _PROG is None:
        _PROG = _build_program()
    return _PROG


def _perm512(a):
    """Permute rows within each 512 block: new position j*128+p <- old 4p+j."""
    n = a.shape[0]
    assert n % 512 == 0
    rest = a.shape[1:]
    return np.ascontiguousarray(
        a.reshape(n // 512, 128, 4, *rest).swapaxes(1, 2).reshape(a.shape)
    )


def _host_inputs(inputs):
    """Build the per-core in_maps from the full problem inputs."""
    g = {k: np.asarray(v) for k, v in inputs.items()}
    sz = np.array([float(g["sz_r"]), float(g["sz_c"])], dtype=np.float64)

    def f32(x):
        return np.ascontiguousarray(np.asarray(x, dtype=np.float32))

    def f16(x):
        return np.ascontiguousarray(np.asarray(x, dtype=np.float16))

    # Mmat [3, 40]: args rows of the scaled matmul (fp16: the 2^k/sz
    # coefficients and the integer idx values are exact in fp16; only the
    # pi/2 shift rounds, by ~5e-4 rad).
    M = np.zeros((3, 40), dtype=np.float64)
    for j in range(40):
        blk = j if j < 20 else j - 20
        k, cc = blk // 2, blk % 2
        M[cc, j] = (2.0**k) / sz[cc]
        M[2, j] = 0.0 if j < 20 else np.pi / 2
    mmat = f16(M)

    Wd1, bd1 = g["Wd1"], g["bd1"]
    Wd2, bd2 = g["Wd2"], g["bd2"]
    consts = {
        "mmat": mmat,
        "w1d": f32(Wd1[:, 0:1]),
        "b1d": f32(bd1.reshape(128, 1)),
        "w2td": f16(np.asarray(Wd2, np.float64).T),
        "b2bd": f32(np.tile(bd2.reshape(1, -1), (128, PC // 128))),
    }

    # posenc weights; X row order -> original pe column order
    sin_src = [2 + 4 * (i // 2) + (i % 2) for i in range(20)]
    cos_src = [2 + 4 * (i // 2) + 2 + (i % 2) for i in range(20)]
    cols = sin_src + cos_src
    for s, W1, b1, W2, b2, W3, b3, tv in (
        ("f", g["Wf1"], g["bf1"], g["Wf2"], g["bf2"], g["Wf3"], g["bf3"], (1.0, 0.0)),
        ("a", g["Wa1"], g["ba1"], g["Wa2"], g["ba2"], g["Wa3"], g["ba3"], (0.0, 1.0)),
    ):
        W1 = np.asarray(W1, np.float64)
        b1e = np.asarray(b1, np.float64) + W1[:, 42] * tv[0] + W1[:, 43] * tv[1]
        consts[f"w1a{s}"] = f16(W1[:, cols].T)               # [40, 64]
        consts[f"w1b{s}"] = f16((W1[:, 0:2] / sz[None, :]).T)  # [2, 64]
        consts[f"b1e{s}"] = f32(b1e.reshape(64, 1))
        consts[f"w2t{s}"] = f16(np.asarray(W2, np.float64).T)  # [64, 128]
        consts[f"b2c{s}"] = f32(np.asarray(b2).reshape(128, 1))
        consts[f"w3t{s}"] = f16(np.asarray(W3, np.float64).T)  # [128, 128]
        consts[f"b3b{s}"] = f32(np.tile(np.asarray(b3).reshape(1, -1), (128, PC // 128)))

    in_maps = []
    fi, gi = g["frontier_idx"], g["ghistory_idx"]
    pi_, ai = g["phistory_idx"], g["agent_pos"]
    dv = g["dist_vals"]
    for b in range(B):
        idxf = np.concatenate([np.asarray(fi[b]), np.asarray(gi[b])], axis=0)
        idxa_raw = np.concatenate([np.asarray(pi_[b]), np.asarray(ai[b])], axis=0)
        idxa = np.zeros((NAB, 2), dtype=np.float64)
        idxa[: idxa_raw.shape[0]] = np.asarray(idxa_raw, np.float64)

        idxf_p = _perm512(np.asarray(idxf, np.float64))
        idxa_p = _perm512(idxa)

        def with_ones(t, n):
            out = np.ones((3, n), dtype=np.float16)
            out[0:2, :] = t.T.astype(np.float16)
            return np.ascontiguousarray(out)

        m = dict(consts)
        m["idxf"] = with_ones(idxf_p, NFB)
        m["idxa"] = with_ones(idxa_p, NAB)
        dperm = _perm512(np.asarray(dv[b], np.float32).reshape(ND))
        m["dist"] = f32(dperm.reshape(128, 512))
        in_maps.append(m)
    return in_maps


def kernel(**inputs):
    global LAST_RESULT, _LEGALIZED
    nc = _get_program()
    if not _LEGALIZED:
        # CoreSim can't execute the injected EventSemaphores, so this runs
        # only on the hardware path.
        _legalize_waits(nc)
        _LEGALIZED = True
    in_maps = _host_inputs(inputs)
    trace = os.environ.get("BASS_TRACE", "") not in ("", "0")
    res = run_bass_kernel_spmd(nc, in_maps, core_ids=list(range(B)), trace=trace)
    LAST_RESULT = res
    r = res.results
    dist_enc = np.stack([r[b]["dist_out"] for b in range(B)])
    frontier = np.stack([r[b]["front_out"] for b in range(B)])
    agent = np.stack([r[b]["agent_out"] for b in range(B)])
    phist = np.stack([r[b]["phist_out"] for b in range(B)])
    ghist = np.stack([r[b]["ghist_out"] for b in range(B)])
    return (dist_enc, frontier, agent, phist, ghist)


# revision 24
# speedup vs baseline: 1.3272x; 1.3272x over previous
"""Trainium2 Bass kernel for nn_Encoder_44238163149250.

Sharding: data-parallel over batch B=8 -> one batch element per NeuronCore.
Each core computes all five encoder outputs for its batch element.

Per-core structure (batch element b), dist and posenc chunks interleaved in
one Tile region so every engine stays loaded across the whole kernel:

  dist branch  (65536 pts, 512-pt chunks):
      y = relu(w1*log1p(d) + b1) @ W2.T + b2
      - log1p once on ACT over the whole [128,512] input; round-trip through
        a DRAM scratch so chunks re-load as [1,512] rows at base partition 0
        (SWDGE casts f32->fp16 during that reload)
      - broadcast across partitions with a K=1 ones outer-product matmul
      - h1 = Relu(scale*psum + bias) on ACT (per-partition scale/bias), fp16
      - layer 2 uses h1 slices as the matmul *stationary* operand
        (lhsT = h1[:,128j], rhs = W2.T fp16) so PSUM comes out in
        [points, feat] layout -> no transpose anywhere
      - bias-add + PSUM->SBUF on DVE; two chunks batched per output DMA
  posenc branches (frontier+ghist = fparams, phist+agent = aparams):
      - freq-scaled args via one K=3 fp16 matmul vs [idx_r; idx_c; 1]
        (integer idx and 2^k/512 coefficients are exact in fp16)
      - range-reduce to [-pi,pi]: k = round(x/2pi) via ACT Identity
        (scale+magic bias) + DVE subtract; r = x - k*fl32(2pi) via DVE
        scalar_tensor_tensor; clamp on DVE; cos rows are sin(x + pi/2)
        with the shift riding the ones row of the idx matmul
      - 3-layer fp16 MLP, layer biases folded into ACT per-partition bias,
        last layer via the same stationary-operand trick

  Point order inside each 512-pt chunk is host-permuted (position j*128+p
  holds point 4p+j) so each partition's DMA store is one contiguous 2KB run
  instead of 4 scattered 512B runs.
"""

import os

import numpy as np

import concourse.bass as bass
import concourse.bass_utils as _bass_utils
import concourse.mybir as mybir
import concourse.tile as tile
from concourse.bass_utils import run_bass_kernel_spmd

# The repo's walrus invocation passes --enable-ldw-opt=false, which leaves
# every matmul paying its LDWEIGHTS serially (~123ns each, ~100us across this
# kernel). Flip it on for our NEFF compiles.
_orig_run_command = _bass_utils.run_command


def _run_command_ldwopt(argv, **kwargs):
    argv = [
        a
        for a in argv
    ]
    return _orig_run_command(argv, **kwargs)


_bass_utils.run_command = _run_command_ldwopt

F32 = mybir.dt.float32
F16 = mybir.dt.float16
AF = mybir.ActivationFunctionType
ALU = mybir.AluOpType

NUM_FREQS = 10
B = 8

# per-core sizes
ND = 65536          # dist points (D*NF = 8*8192)
NFR = 8192          # frontier pts
NG = 2048           # ghist pts
NPH = 2048          # phist pts
NA = 8              # agent pts
NFB = NFR + NG      # F-branch pts  (fparams + type_frontier)
NAB = 2560          # A-branch pts padded (phist 2048 + agent 8 -> 2560)

PC = 512            # chunk size (points) for both branches
NDC = ND // PC      # 128 dist chunks
NFC = NFB // PC     # 20
NAC = NAB // PC     # 5

MAGIC = 12582912.0                       # 1.5 * 2**23: round-to-nearest trick
TWO_PI = 2.0 * np.pi
FL2PI = float(np.float32(TWO_PI))        # r = x - k*fl32(2pi): |err| <= 5e-5 rad
INV_2PI = 1.0 / TWO_PI
PI_LO = float(np.nextafter(np.float32(np.pi), np.float32(0)))  # f32 just below pi

_PROG = None        # cached nc build
_LEGALIZED = False  # wait-legalization applied to _PROG (HW path only)
LAST_RESULT = None  # BassKernelResults of the last kernel() call (for test.py)

DIST_ON = os.environ.get("K_DIST", "1") == "1"
POSENC_ON = os.environ.get("K_POSENC", "1") == "1"


def _legalize_waits(nc):
    """This toolchain's walrus accepts at most ONE sync wait per instruction
    (codegen raises 'Too many sync wait commands' otherwise), while Tile
    freely emits several. Hoist all but the last wait of each instruction
    onto same-engine EventSemaphore instructions inserted right before it —
    engines execute their queue in order, so sequential waits are equivalent
    to one multi-wait."""
    n = 0
    for f in nc.m.functions:
        for b in f.blocks:
            insts = list(b.instructions)
            out = []
            changed = False
            for i in insts:
                si = i.sync_info
                if si is not None and len(si.on_wait) > 1:
                    waits = list(si.on_wait)
                    for j, w in enumerate(waits[:-1]):
                        ev = mybir.InstEventSemaphore(
                            name=f"lw_{i.name}_{j}",
                            engine=i.engine,
                            ins=[],
                            outs=[],
                            sync_info=mybir.SyncInfo(on_wait=[w], on_update=[]),
                        )
                        out.append(ev)
                        n += 1
                    i.sync_info = mybir.SyncInfo(
                        on_wait=[waits[-1]], on_update=list(si.on_update)
                    )
                    changed = True
                out.append(i)
            if changed:
                cur = b.instructions
                del cur[:]
                for i in out:
                    cur.append(i)
    return n


def _build_program():
    nc = bass.Bass()

    # ---- DRAM I/O ----
    din = nc.dram_tensor("dist", [128, 512], F32, kind="ExternalInput")
    idx_f = nc.dram_tensor("idxf", [3, NFB], F16, kind="ExternalInput")
    idx_a = nc.dram_tensor("idxa", [3, NAB], F16, kind="ExternalInput")
    mmat = nc.dram_tensor("mmat", [3, 40], F16, kind="ExternalInput")
    w1d = nc.dram_tensor("w1d", [128, 1], F32, kind="ExternalInput")
    b1d = nc.dram_tensor("b1d", [128, 1], F32, kind="ExternalInput")
    w2td = nc.dram_tensor("w2td", [128, 128], F16, kind="ExternalInput")
    b2bd = nc.dram_tensor("b2bd", [128, PC], F32, kind="ExternalInput")
    brt = {}
    for s in ("f", "a"):
        brt[s] = dict(
            w1a=nc.dram_tensor(f"w1a{s}", [40, 64], F16, kind="ExternalInput"),
            w1b=nc.dram_tensor(f"w1b{s}", [2, 64], F16, kind="ExternalInput"),
            b1e=nc.dram_tensor(f"b1e{s}", [64, 1], F32, kind="ExternalInput"),
            w2t=nc.dram_tensor(f"w2t{s}", [64, 128], F16, kind="ExternalInput"),
            b2c=nc.dram_tensor(f"b2c{s}", [128, 1], F32, kind="ExternalInput"),
            w3t=nc.dram_tensor(f"w3t{s}", [128, 128], F16, kind="ExternalInput"),
            b3b=nc.dram_tensor(f"b3b{s}", [128, PC], F32, kind="ExternalInput"),
        )

    dout = nc.dram_tensor("dist_out", [ND, 128], F32, kind="ExternalOutput")
    fout = nc.dram_tensor("front_out", [NFR, 128], F32, kind="ExternalOutput")
    gout = nc.dram_tensor("ghist_out", [NG, 128], F32, kind="ExternalOutput")
    pout = nc.dram_tensor("phist_out", [NPH, 128], F32, kind="ExternalOutput")
    aout = nc.dram_tensor("agent_out", [NA, 128], F32, kind="ExternalOutput")

    # Views matching the in-chunk permutation: position j*128+p <-> point
    # 512c + 4p + j. Dist pairs two chunks per DMA.
    dview = dout.rearrange("(t h p g) o -> t p h g o", h=2, p=128, g=4)
    fview = fout.rearrange("(c p g) o -> c p g o", p=128, g=4)
    gview = gout.rearrange("(c p g) o -> c p g o", p=128, g=4)
    pview = pout.rearrange("(c p g) o -> c p g o", p=128, g=4)
    aview = aout.rearrange("(p j) o -> p j o", j=4)

    with tile.TileContext(nc) as tc:
        with tc.tile_pool(name="const", bufs=1) as cp:

            def cload(dram, shape, tag, dt=F32):
                t = cp.tile(shape, dt, tag=tag)
                nc.sync.dma_start(out=t[:], in_=dram[:, :])
                return t

            w1d_t = cload(w1d, [128, 1], "w1d")
            b1d_t = cload(b1d, [128, 1], "b1d")
            w2td_t = cload(w2td, [128, 128], "w2td", F16)
            b2bd_t = cload(b2bd, [128, PC], "b2bd")
            mmat_t = cload(mmat, [3, 40], "mmat", F16)
            brc = {}
            for s in ("f", "a"):
                d = brt[s]
                brc[s] = dict(
                    w1a=cload(d["w1a"], [40, 64], f"w1a{s}", F16),
                    w1b=cload(d["w1b"], [2, 64], f"w1b{s}", F16),
                    b1e=cload(d["b1e"], [64, 1], f"b1e{s}"),
                    w2t=cload(d["w2t"], [64, 128], f"w2t{s}", F16),
                    b2c=cload(d["b2c"], [128, 1], f"b2c{s}"),
                    w3t=cload(d["w3t"], [128, 128], f"w3t{s}", F16),
                    b3b=cload(d["b3b"], [128, PC], f"b3b{s}"),
                )
            idxf_t = cload(idx_f, [3, NFB], "idxf", F16)
            idxa_t = cload(idx_a, [3, NAB], "idxa", F16)

            ones_l = cp.tile([1, 128], F16, tag="ones")
            nc.vector.memset(ones_l[:], 1.0)
            magic_c = cp.tile([40, 1], F32, tag="magic")
            nc.vector.memset(magic_c[:], MAGIC)

            # HAM warmup: the PE clock-gate defaults to K=4/8 (1.2 GHz) and
            # only reaches 2.4 GHz after ~3.4us of *continuous* matmul
            # activity. This kernel's real matmul bursts are ~0.9us, so
            # without a warmup the whole kernel streams at half clock
            # (measured: zero HAM transitions, N=128 MMs at 107ns spacing).
            # Burn ~5us of back-to-back K=1 matmuls on a scratch bank first;
            # afterwards no PE gap exceeds the ~3.4us re-throttle window.
            warm_r = cp.tile([128, 512], F16, tag="warm_r")
            nc.vector.memset(warm_r[:], 1.0)
            with tc.tile_pool(name="warmp", bufs=1, space="PSUM") as warmp:
                wps = warmp.tile([128, 512], F32, tag="wps")
                for _ in range(14):
                    nc.tensor.matmul(
                        wps[:], lhsT=warm_r[:, 0:128], rhs=warm_r[:],
                        start=True, stop=True,
                    )

            # dist input + log1p, once (input already host-permuted)
            dist_sb = cp.tile([128, 512], F32, tag="dist_sb")
            nc.sync.dma_start(out=dist_sb[:], in_=din[:, :])
            logd = cp.tile([128, 512], F32, tag="logd")
            nc.scalar.activation(logd[:], dist_sb[:], AF.Ln, bias=1.0, scale=1.0)

            with (
                tc.tile_pool(name="ddram", bufs=1, space="DRAM") as ddram,
                tc.tile_pool(name="dsb", bufs=4) as dsb,
                tc.tile_pool(name="esb", bufs=4) as esb,
                tc.tile_pool(name="dpb", bufs=2, space="PSUM") as dpb,
                tc.tile_pool(name="dpo", bufs=2, space="PSUM") as dpo,
                tc.tile_pool(name="eps", bufs=1, space="PSUM") as eps,
                tc.tile_pool(name="ep1", bufs=1, space="PSUM") as ep1,
                tc.tile_pool(name="ep2", bufs=1, space="PSUM") as ep2,
                tc.tile_pool(name="epo", bufs=1, space="PSUM") as epo,
            ):
                scr = ddram.tile([128, 512], F32, tag="scr")
                nc.gpsimd.dma_start(out=scr[:], in_=logd[:])

                def dist_chunk(c, ot_half):
                    ct = dsb.tile([1, PC], F16, tag="ct")
                    nc.gpsimd.dma_start(out=ct[:], in_=scr[c : c + 1, :])
                    psb = dpb.tile([128, PC], F32, tag="psb")
                    nc.tensor.matmul(
                        psb[:], lhsT=ones_l[:], rhs=ct[0:1, :],
                        start=True, stop=True,
                    )
                    h1 = dsb.tile([128, PC], F16, tag="h1")
                    nc.scalar.activation(
                        h1[:], psb[:], AF.Relu, bias=b1d_t[:], scale=w1d_t[:]
                    )
                    pso = dpo.tile([128, PC], F32, tag="pso")
                    for j in range(PC // 128):
                        nc.tensor.matmul(
                            pso[:, j * 128 : (j + 1) * 128],
                            lhsT=h1[:, j * 128 : (j + 1) * 128],
                            rhs=w2td_t[:],
                            start=True,
                            stop=True,
                        )
                    nc.vector.tensor_add(ot_half, pso[:], b2bd_t[:])

                def posenc_chunk(s, idx_t, c):
                    w = brc[s]
                    lo = c * PC
                    pss = eps.tile([40, PC], F32, tag="pss")
                    nc.tensor.matmul(
                        pss[:], lhsT=mmat_t[:], rhs=idx_t[:, lo : lo + PC],
                        start=True, stop=True,
                    )
                    # k = round(args/2pi): scale+magic on ACT, subtract on DVE
                    kt1 = esb.tile([40, PC], F32, tag="kt1")
                    nc.scalar.activation(
                        kt1[:], pss[:], AF.Identity,
                        bias=magic_c[:], scale=INV_2PI,
                    )
                    kt = esb.tile([40, PC], F32, tag="kt")
                    nc.vector.tensor_scalar(
                        out=kt[:], in0=kt1[:], scalar1=MAGIC, scalar2=None,
                        op0=ALU.subtract,
                    )
                    rt = esb.tile([40, PC], F32, tag="rt")
                    nc.vector.scalar_tensor_tensor(
                        out=rt[:], in0=kt[:], scalar=-FL2PI,
                        in1=pss[:], op0=ALU.mult, op1=ALU.add,
                    )
                    r3 = esb.tile([40, PC], F32, tag="r3")
                    nc.vector.tensor_scalar(
                        out=r3[:], in0=rt[:],
                        scalar1=PI_LO, scalar2=-PI_LO,
                        op0=ALU.min, op1=ALU.max,
                    )
                    xs = esb.tile([40, PC], F16, tag="xs")
                    nc.scalar.activation(xs[:], r3[:], AF.Sin)

                    ps1 = ep1.tile([64, PC], F32, tag="ps1")
                    nc.tensor.matmul(
                        ps1[:], lhsT=w["w1a"][:], rhs=xs[:],
                        start=True, stop=False,
                    )
                    nc.tensor.matmul(
                        ps1[:], lhsT=w["w1b"][:], rhs=idx_t[0:2, lo : lo + PC],
                        start=False, stop=True,
                    )
                    h1e = esb.tile([64, PC], F16, tag="h1e")
                    nc.scalar.activation(h1e[:], ps1[:], AF.Relu, bias=w["b1e"][:])

                    ps2 = ep2.tile([128, PC], F32, tag="ps2")
                    nc.tensor.matmul(
                        ps2[:], lhsT=w["w2t"][:], rhs=h1e[:], start=True, stop=True
                    )
                    h2e = esb.tile([128, PC], F16, tag="h2e")
                    nc.scalar.activation(h2e[:], ps2[:], AF.Relu, bias=w["b2c"][:])

                    po = epo.tile([128, PC], F32, tag="po")
                    for j in range(PC // 128):
                        nc.tensor.matmul(
                            po[:, j * 128 : (j + 1) * 128],
                            lhsT=h2e[:, j * 128 : (j + 1) * 128],
                            rhs=w["w3t"][:],
                            start=True,
                            stop=True,
                        )
                    oute = esb.tile([128, PC], F32, tag="oute")
                    nc.vector.tensor_add(oute[:], po[:], w["b3b"][:])

                    src = oute[:].rearrange("p (g o) -> p g o", g=4)
                    if s == "f":
                        if c < NFR // PC:
                            nc.sync.dma_start(out=fview[c, :, :, :], in_=src)
                        else:
                            nc.sync.dma_start(
                                out=gview[c - NFR // PC, :, :, :], in_=src
                            )
                    else:
                        if c < NPH // PC:
                            nc.sync.dma_start(out=pview[c, :, :, :], in_=src)
                        else:
                            nc.sync.dma_start(
                                out=aview[:, :, :],
                                in_=oute[0:2, :].rearrange("p (j o) -> p j o", j=4),
                            )

                pe_jobs = []
                if POSENC_ON:
                    pe_jobs = [("f", idxf_t, c) for c in range(NFC)] + [
                        ("a", idxa_t, c) for c in range(NAC)
                    ]
                n_pairs = NDC // 2 if DIST_ON else 0
                emitted = 0
                for t in range(n_pairs):
                    ot = dsb.tile([128, 2 * PC], F32, tag="ot")
                    for h in range(2):
                        dist_chunk(2 * t + h, ot[:, h * PC : (h + 1) * PC])
                    nc.sync.dma_start(
                        out=dview[t, :, :, :, :],
                        in_=ot[:].rearrange("p (h g o) -> p h g o", h=2, g=4),
                    )
                    want = (t + 1) * len(pe_jobs) // n_pairs
                    while emitted < want:
                        posenc_chunk(*pe_jobs[emitted])
                        emitted += 1
                while emitted < len(pe_jobs):
                    posenc_chunk(*pe_jobs[emitted])
                    emitted += 1
    return nc


def _get_program():
    global _PROG
    if _PROG is None:
        _PROG = _build_program()
    return _PROG


def _perm512(a):
    """Permute rows within each 512 block: new position j*128+p <- old 4p+j."""
    n = a.shape[0]
    assert n % 512 == 0
    rest = a.shape[1:]
    return np.ascontiguousarray(
        a.reshape(n // 512, 128, 4, *rest).swapaxes(1, 2).reshape(a.shape)
    )


def _host_inputs(inputs):
    """Build the per-core in_maps from the full problem inputs."""
    g = {k: np.asarray(v) for k, v in inputs.items()}
    sz = np.array([float(g["sz_r"]), float(g["sz_c"])], dtype=np.float64)

    def f32(x):
        return np.ascontiguousarray(np.asarray(x, dtype=np.float32))

    def f16(x):
        return np.ascontiguousarray(np.asarray(x, dtype=np.float16))

    # Mmat [3, 40]: args rows of the scaled matmul (fp16: the 2^k/sz
    # coefficients and the integer idx values are exact in fp16; only the
    # pi/2 shift rounds, by ~5e-4 rad).
    M = np.zeros((3, 40), dtype=np.float64)
    for j in range(40):
        blk = j if j < 20 else j - 20
        k, cc = blk // 2, blk % 2
        M[cc, j] = (2.0**k) / sz[cc]
        M[2, j] = 0.0 if j < 20 else np.pi / 2
    mmat = f16(M)

    Wd1, bd1 = g["Wd1"], g["bd1"]
    Wd2, bd2 = g["Wd2"], g["bd2"]
    consts = {
        "mmat": mmat,
        "w1d": f32(Wd1[:, 0:1]),
        "b1d": f32(bd1.reshape(128, 1)),
        "w2td": f16(np.asarray(Wd2, np.float64).T),
        "b2bd": f32(np.tile(bd2.reshape(1, -1), (128, PC // 128))),
    }

    # posenc weights; X row order -> original pe column order
    sin_src = [2 + 4 * (i // 2) + (i % 2) for i in range(20)]
    cos_src = [2 + 4 * (i // 2) + 2 + (i % 2) for i in range(20)]
    cols = sin_src + cos_src
    for s, W1, b1, W2, b2, W3, b3, tv in (
        ("f", g["Wf1"], g["bf1"], g["Wf2"], g["bf2"], g["Wf3"], g["bf3"], (1.0, 0.0)),
        ("a", g["Wa1"], g["ba1"], g["Wa2"], g["ba2"], g["Wa3"], g["ba3"], (0.0, 1.0)),
    ):
        W1 = np.asarray(W1, np.float64)
        b1e = np.asarray(b1, np.float64) + W1[:, 42] * tv[0] + W1[:, 43] * tv[1]
        consts[f"w1a{s}"] = f16(W1[:, cols].T)               # [40, 64]
        consts[f"w1b{s}"] = f16((W1[:, 0:2] / sz[None, :]).T)  # [2, 64]
        consts[f"b1e{s}"] = f32(b1e.reshape(64, 1))
        consts[f"w2t{s}"] = f16(np.asarray(W2, np.float64).T)  # [64, 128]
        consts[f"b2c{s}"] = f32(np.asarray(b2).reshape(128, 1))
        consts[f"w3t{s}"] = f16(np.asarray(W3, np.float64).T)  # [128, 128]
        consts[f"b3b{s}"] = f32(np.tile(np.asarray(b3).reshape(1, -1), (128, PC // 128)))

    in_maps = []
    fi, gi = g["frontier_idx"], g["ghistory_idx"]
    pi_, ai = g["phistory_idx"], g["agent_pos"]
    dv = g["dist_vals"]
    for b in range(B):
        idxf = np.concatenate([np.asarray(fi[b]), np.asarray(gi[b])], axis=0)
        idxa_raw = np.concatenate([np.asarray(pi_[b]), np.asarray(ai[b])], axis=0)
        idxa = np.zeros((NAB, 2), dtype=np.float64)
        idxa[: idxa_raw.shape[0]] = np.asarray(idxa_raw, np.float64)

        idxf_p = _perm512(np.asarray(idxf, np.float64))
        idxa_p = _perm512(idxa)

        def with_ones(t, n):
            out = np.ones((3, n), dtype=np.float16)
            out[0:2, :] = t.T.astype(np.float16)
            return np.ascontiguousarray(out)

        m = dict(consts)
        m["idxf"] = with_ones(idxf_p, NFB)
        m["idxa"] = with_ones(idxa_p, NAB)
        dperm = _perm512(np.asarray(dv[b], np.float32).reshape(ND))
        m["dist"] = f32(dperm.reshape(128, 512))
        in_maps.append(m)
    return in_maps


def kernel(**inputs):
    global LAST_RESULT, _LEGALIZED
    nc = _get_program()
    if not _LEGALIZED:
        # CoreSim can't execute the injected EventSemaphores, so this runs
        # only on the hardware path.
        _legalize_waits(nc)
        _LEGALIZED = True
    in_maps = _host_inputs(inputs)
    trace = os.environ.get("BASS_TRACE", "") not in ("", "0")
    res = run_bass_kernel_spmd(nc, in_maps, core_ids=list(range(B)), trace=trace)
    LAST_RESULT = res
    r = res.results
    dist_enc = np.stack([r[b]["dist_out"] for b in range(B)])
    frontier = np.stack([r[b]["front_out"] for b in range(B)])
    agent = np.stack([r[b]["agent_out"] for b in range(B)])
    phist = np.stack([r[b]["phist_out"] for b in range(B)])
    ghist = np.stack([r[b]["ghist_out"] for b in range(B)])
    return (dist_enc, frontier, agent, phist, ghist)


# revision 26
# speedup vs baseline: 1.3825x; 1.0416x over previous
"""Trainium2 Bass kernel for nn_Encoder_44238163149250.

Sharding: data-parallel over batch B=8 -> one batch element per NeuronCore.
Each core computes all five encoder outputs for its batch element.

Per-core structure (batch element b), dist and posenc chunks interleaved in
one Tile region so every engine stays loaded across the whole kernel:

  dist branch  (65536 pts, 512-pt chunks):
      y = relu(w1*log1p(d) + b1) @ W2.T + b2
      - log1p once on ACT over the whole [128,512] input; round-trip through
        a DRAM scratch so chunks re-load as [1,512] rows at base partition 0
        (SWDGE casts f32->fp16 during that reload)
      - broadcast across partitions with a K=1 ones outer-product matmul
      - h1 = Relu(scale*psum + bias) on ACT (per-partition scale/bias), fp16
      - layer 2 uses h1 slices as the matmul *stationary* operand
        (lhsT = h1[:,128j], rhs = W2.T fp16) so PSUM comes out in
        [points, feat] layout -> no transpose anywhere
      - bias-add + PSUM->SBUF on DVE; two chunks batched per output DMA
  posenc branches (frontier+ghist = fparams, phist+agent = aparams):
      - freq-scaled args via one K=3 fp16 matmul vs [idx_r; idx_c; 1]
        (integer idx and 2^k/512 coefficients are exact in fp16)
      - range-reduce to [-pi,pi]: k = round(x/2pi) via ACT Identity
        (scale+magic bias) + DVE subtract; r = x - k*fl32(2pi) via DVE
        scalar_tensor_tensor; clamp on DVE; cos rows are sin(x + pi/2)
        with the shift riding the ones row of the idx matmul
      - 3-layer fp16 MLP, layer biases folded into ACT per-partition bias,
        last layer via the same stationary-operand trick

  Point order inside each 512-pt chunk is host-permuted (position j*128+p
  holds point 4p+j) so each partition's DMA store is one contiguous 2KB run
  instead of 4 scattered 512B runs.
"""

import os

import numpy as np

import concourse.bass as bass
import concourse.bass_utils as _bass_utils
import concourse.mybir as mybir
import concourse.tile as tile
from concourse.bass_utils import run_bass_kernel_spmd

# The repo's walrus invocation passes --enable-ldw-opt=false, which leaves
# every matmul paying its LDWEIGHTS serially (~123ns each, ~100us across this
# kernel). Flip it on for our NEFF compiles.
_orig_run_command = _bass_utils.run_command


def _run_command_ldwopt(argv, **kwargs):
    argv = [
        a
        for a in argv
    ]
    return _orig_run_command(argv, **kwargs)


_bass_utils.run_command = _run_command_ldwopt

F32 = mybir.dt.float32
F16 = mybir.dt.float16
AF = mybir.ActivationFunctionType
ALU = mybir.AluOpType

NUM_FREQS = 10
B = 8

# per-core sizes
ND = 65536          # dist points (D*NF = 8*8192)
NFR = 8192          # frontier pts
NG = 2048           # ghist pts
NPH = 2048          # phist pts
NA = 8              # agent pts
NFB = NFR + NG      # F-branch pts  (fparams + type_frontier)
NAB = 2560          # A-branch pts padded (phist 2048 + agent 8 -> 2560)

PC = 512            # chunk size (points) for both branches
NDC = ND // PC      # 128 dist chunks
NFC = NFB // PC     # 20
NAC = NAB // PC     # 5

MAGIC = 12582912.0                       # 1.5 * 2**23: round-to-nearest trick
TWO_PI = 2.0 * np.pi
FL2PI = float(np.float32(TWO_PI))        # r = x - k*fl32(2pi): |err| <= 5e-5 rad
INV_2PI = 1.0 / TWO_PI
PI_LO = float(np.nextafter(np.float32(np.pi), np.float32(0)))  # f32 just below pi

_PROG = None        # cached nc build
_LEGALIZED = False  # wait-legalization applied to _PROG (HW path only)
LAST_RESULT = None  # BassKernelResults of the last kernel() call (for test.py)

DIST_ON = os.environ.get("K_DIST", "1") == "1"
POSENC_ON = os.environ.get("K_POSENC", "1") == "1"


def _legalize_waits(nc):
    """This toolchain's walrus accepts at most ONE sync wait per instruction
    (codegen raises 'Too many sync wait commands' otherwise), while Tile
    freely emits several. Hoist all but the last wait of each instruction
    onto same-engine EventSemaphore instructions inserted right before it —
    engines execute their queue in order, so sequential waits are equivalent
    to one multi-wait."""
    n = 0
    for f in nc.m.functions:
        for b in f.blocks:
            insts = list(b.instructions)
            out = []
            changed = False
            for i in insts:
                si = i.sync_info
                if si is not None and len(si.on_wait) > 1:
                    waits = list(si.on_wait)
                    for j, w in enumerate(waits[:-1]):
                        ev = mybir.InstEventSemaphore(
                            name=f"lw_{i.name}_{j}",
                            engine=i.engine,
                            ins=[],
                            outs=[],
                            sync_info=mybir.SyncInfo(on_wait=[w], on_update=[]),
                        )
                        out.append(ev)
                        n += 1
                    i.sync_info = mybir.SyncInfo(
                        on_wait=[waits[-1]], on_update=list(si.on_update)
                    )
                    changed = True
                out.append(i)
            if changed:
                cur = b.instructions
                del cur[:]
                for i in out:
                    cur.append(i)
    return n


def _build_program():
    nc = bass.Bass()

    # ---- DRAM I/O ----
    din = nc.dram_tensor("dist", [128, 512], F32, kind="ExternalInput")
    idx_f = nc.dram_tensor("idxf", [3, NFB], F16, kind="ExternalInput")
    idx_a = nc.dram_tensor("idxa", [3, NAB], F16, kind="ExternalInput")
    mmat = nc.dram_tensor("mmat", [3, 40], F16, kind="ExternalInput")
    w1d = nc.dram_tensor("w1d", [128, 1], F32, kind="ExternalInput")
    b1d = nc.dram_tensor("b1d", [128, 1], F32, kind="ExternalInput")
    w2td = nc.dram_tensor("w2td", [128, 128], F16, kind="ExternalInput")
    b2bd = nc.dram_tensor("b2bd", [128, PC], F32, kind="ExternalInput")
    brt = {}
    for s in ("f", "a"):
        brt[s] = dict(
            w1a=nc.dram_tensor(f"w1a{s}", [40, 64], F16, kind="ExternalInput"),
            w1b=nc.dram_tensor(f"w1b{s}", [2, 64], F16, kind="ExternalInput"),
            b1e=nc.dram_tensor(f"b1e{s}", [64, 1], F32, kind="ExternalInput"),
            w2t=nc.dram_tensor(f"w2t{s}", [64, 128], F16, kind="ExternalInput"),
            b2c=nc.dram_tensor(f"b2c{s}", [128, 1], F32, kind="ExternalInput"),
            w3t=nc.dram_tensor(f"w3t{s}", [128, 128], F16, kind="ExternalInput"),
            b3b=nc.dram_tensor(f"b3b{s}", [128, PC], F32, kind="ExternalInput"),
        )

    dout = nc.dram_tensor("dist_out", [ND, 128], F16, kind="ExternalOutput")
    fout = nc.dram_tensor("front_out", [NFR, 128], F16, kind="ExternalOutput")
    gout = nc.dram_tensor("ghist_out", [NG, 128], F16, kind="ExternalOutput")
    pout = nc.dram_tensor("phist_out", [NPH, 128], F16, kind="ExternalOutput")
    aout = nc.dram_tensor("agent_out", [NA, 128], F16, kind="ExternalOutput")

    # Views matching the in-chunk permutation: position j*128+p <-> point
    # 512c + 4p + j. Dist pairs two chunks per DMA.
    dview = dout.rearrange("(t h p g) o -> t p h g o", h=2, p=128, g=4)
    fview = fout.rearrange("(c p g) o -> c p g o", p=128, g=4)
    gview = gout.rearrange("(c p g) o -> c p g o", p=128, g=4)
    pview = pout.rearrange("(c p g) o -> c p g o", p=128, g=4)
    aview = aout.rearrange("(p j) o -> p j o", j=4)

    with tile.TileContext(nc) as tc:
        with tc.tile_pool(name="const", bufs=1) as cp:
            # dist input + log1p first so the scratch/ct chain starts before
            # the (many) constant loads occupy the queues
            dist_sb = cp.tile([128, 512], F32, tag="dist_sb")
            nc.sync.dma_start(out=dist_sb[:], in_=din[:, :])
            logd = cp.tile([128, 512], F32, tag="logd")
            nc.scalar.activation(logd[:], dist_sb[:], AF.Ln, bias=1.0, scale=1.0)

            def cload(dram, shape, tag, dt=F32):
                t = cp.tile(shape, dt, tag=tag)
                nc.sync.dma_start(out=t[:], in_=dram[:, :])
                return t

            w1d_t = cload(w1d, [128, 1], "w1d")
            b1d_t = cload(b1d, [128, 1], "b1d")
            w2td_t = cload(w2td, [128, 128], "w2td", F16)
            b2bd_t = cload(b2bd, [128, PC], "b2bd")
            mmat_t = cload(mmat, [3, 40], "mmat", F16)
            brc = {}
            for s in ("f", "a"):
                d = brt[s]
                brc[s] = dict(
                    w1a=cload(d["w1a"], [40, 64], f"w1a{s}", F16),
                    w1b=cload(d["w1b"], [2, 64], f"w1b{s}", F16),
                    b1e=cload(d["b1e"], [64, 1], f"b1e{s}"),
                    w2t=cload(d["w2t"], [64, 128], f"w2t{s}", F16),
                    b2c=cload(d["b2c"], [128, 1], f"b2c{s}"),
                    w3t=cload(d["w3t"], [128, 128], f"w3t{s}", F16),
                    b3b=cload(d["b3b"], [128, PC], f"b3b{s}"),
                )
            idxf_t = cload(idx_f, [3, NFB], "idxf", F16)
            idxa_t = cload(idx_a, [3, NAB], "idxa", F16)

            ones_l = cp.tile([1, 128], F16, tag="ones")
            nc.vector.memset(ones_l[:], 1.0)
            magic_c = cp.tile([40, 1], F32, tag="magic")
            nc.vector.memset(magic_c[:], MAGIC)

            # HAM warmup: the PE clock-gate defaults to K=4/8 (1.2 GHz) and
            # only reaches 2.4 GHz after ~3.4us of *continuous* matmul
            # activity. This kernel's real matmul bursts are ~0.9us, so
            # without a warmup the whole kernel streams at half clock
            # (measured: zero HAM transitions, N=128 MMs at 107ns spacing).
            # Burn ~5us of back-to-back K=1 matmuls on a scratch bank first;
            # afterwards no PE gap exceeds the ~3.4us re-throttle window.
            warm_r = cp.tile([128, 512], F16, tag="warm_r")
            nc.vector.memset(warm_r[:], 1.0)
            with tc.tile_pool(name="warmp", bufs=1, space="PSUM") as warmp:
                wps = warmp.tile([128, 512], F32, tag="wps")
                for _ in range(48):
                    nc.tensor.matmul(
                        wps[:], lhsT=warm_r[:, 0:128], rhs=warm_r[:],
                        start=True, stop=True,
                    )


            with (
                tc.tile_pool(name="ddram", bufs=1, space="DRAM") as ddram,
                tc.tile_pool(name="dsb", bufs=4) as dsb,
                tc.tile_pool(name="esb", bufs=4) as esb,
                tc.tile_pool(name="dpb", bufs=2, space="PSUM") as dpb,
                tc.tile_pool(name="dpo", bufs=2, space="PSUM") as dpo,
                tc.tile_pool(name="eps", bufs=1, space="PSUM") as eps,
                tc.tile_pool(name="ep1", bufs=1, space="PSUM") as ep1,
                tc.tile_pool(name="ep2", bufs=1, space="PSUM") as ep2,
                tc.tile_pool(name="epo", bufs=1, space="PSUM") as epo,
            ):
                scr = ddram.tile([128, 512], F32, tag="scr")
                nc.gpsimd.dma_start(out=scr[:], in_=logd[:])

                def dist_chunk(c, ot_half):
                    ct = dsb.tile([1, PC], F16, tag="ct")
                    nc.gpsimd.dma_start(out=ct[:], in_=scr[c : c + 1, :])
                    psb = dpb.tile([128, PC], F32, tag="psb")
                    nc.tensor.matmul(
                        psb[:], lhsT=ones_l[:], rhs=ct[0:1, :],
                        start=True, stop=True,
                    )
                    h1 = dsb.tile([128, PC], F16, tag="h1")
                    nc.scalar.activation(
                        h1[:], psb[:], AF.Relu, bias=b1d_t[:], scale=w1d_t[:]
                    )
                    pso = dpo.tile([128, PC], F32, tag="pso")
                    for j in range(PC // 128):
                        nc.tensor.matmul(
                            pso[:, j * 128 : (j + 1) * 128],
                            lhsT=h1[:, j * 128 : (j + 1) * 128],
                            rhs=w2td_t[:],
                            start=True,
                            stop=True,
                        )
                    nc.vector.tensor_add(ot_half, pso[:], b2bd_t[:])

                def posenc_chunk(s, idx_t, c):
                    w = brc[s]
                    lo = c * PC
                    pss = eps.tile([40, PC], F32, tag="pss")
                    nc.tensor.matmul(
                        pss[:], lhsT=mmat_t[:], rhs=idx_t[:, lo : lo + PC],
                        start=True, stop=True,
                    )
                    # k = round(args/2pi): scale+magic on ACT, subtract on DVE
                    kt1 = esb.tile([40, PC], F32, tag="kt1")
                    nc.scalar.activation(
                        kt1[:], pss[:], AF.Identity,
                        bias=magic_c[:], scale=INV_2PI,
                    )
                    kt = esb.tile([40, PC], F32, tag="kt")
                    nc.vector.tensor_scalar(
                        out=kt[:], in0=kt1[:], scalar1=MAGIC, scalar2=None,
                        op0=ALU.subtract,
                    )
                    rt = esb.tile([40, PC], F32, tag="rt")
                    nc.vector.scalar_tensor_tensor(
                        out=rt[:], in0=kt[:], scalar=-FL2PI,
                        in1=pss[:], op0=ALU.mult, op1=ALU.add,
                    )
                    r3 = esb.tile([40, PC], F32, tag="r3")
                    nc.vector.tensor_scalar(
                        out=r3[:], in0=rt[:],
                        scalar1=PI_LO, scalar2=-PI_LO,
                        op0=ALU.min, op1=ALU.max,
                    )
                    xs = esb.tile([40, PC], F16, tag="xs")
                    nc.scalar.activation(xs[:], r3[:], AF.Sin)

                    ps1 = ep1.tile([64, PC], F32, tag="ps1")
                    nc.tensor.matmul(
                        ps1[:], lhsT=w["w1a"][:], rhs=xs[:],
                        start=True, stop=False,
                    )
                    nc.tensor.matmul(
                        ps1[:], lhsT=w["w1b"][:], rhs=idx_t[0:2, lo : lo + PC],
                        start=False, stop=True,
                    )
                    h1e = esb.tile([64, PC], F16, tag="h1e")
                    nc.scalar.activation(h1e[:], ps1[:], AF.Relu, bias=w["b1e"][:])

                    ps2 = ep2.tile([128, PC], F32, tag="ps2")
                    nc.tensor.matmul(
                        ps2[:], lhsT=w["w2t"][:], rhs=h1e[:], start=True, stop=True
                    )
                    h2e = esb.tile([128, PC], F16, tag="h2e")
                    nc.scalar.activation(h2e[:], ps2[:], AF.Relu, bias=w["b2c"][:])

                    po = epo.tile([128, PC], F32, tag="po")
                    for j in range(PC // 128):
                        nc.tensor.matmul(
                            po[:, j * 128 : (j + 1) * 128],
                            lhsT=h2e[:, j * 128 : (j + 1) * 128],
                            rhs=w["w3t"][:],
                            start=True,
                            stop=True,
                        )
                    oute = esb.tile([128, PC], F16, tag="oute")
                    nc.vector.tensor_add(oute[:], po[:], w["b3b"][:])

                    src = oute[:].rearrange("p (g o) -> p g o", g=4)
                    if s == "f":
                        if c < NFR // PC:
                            nc.sync.dma_start(out=fview[c, :, :, :], in_=src)
                        else:
                            nc.sync.dma_start(
                                out=gview[c - NFR // PC, :, :, :], in_=src
                            )
                    else:
                        if c < NPH // PC:
                            nc.sync.dma_start(out=pview[c, :, :, :], in_=src)
                        else:
                            nc.sync.dma_start(
                                out=aview[:, :, :],
                                in_=oute[0:2, :].rearrange("p (j o) -> p j o", j=4),
                            )

                pe_jobs = []
                if POSENC_ON:
                    pe_jobs = [("f", idxf_t, c) for c in range(NFC)] + [
                        ("a", idxa_t, c) for c in range(NAC)
                    ]
                n_pairs = NDC // 2 if DIST_ON else 0
                emitted = 0
                for t in range(n_pairs):
                    ot = dsb.tile([128, 2 * PC], F16, tag="ot")
                    for h in range(2):
                        dist_chunk(2 * t + h, ot[:, h * PC : (h + 1) * PC])
                    nc.sync.dma_start(
                        out=dview[t, :, :, :, :],
                        in_=ot[:].rearrange("p (h g o) -> p h g o", h=2, g=4),
                    )
                    want = (t + 1) * len(pe_jobs) // n_pairs
                    while emitted < want:
                        posenc_chunk(*pe_jobs[emitted])
                        emitted += 1
                while emitted < len(pe_jobs):
                    posenc_chunk(*pe_jobs[emitted])
                    emitted += 1
    return nc


def _get_program():
    global _PROG
    if _PROG is None:
        _PROG = _build_program()
    return _PROG


def _perm512(a):
    """Permute rows within each 512 block: new position j*128+p <- old 4p+j."""
    n = a.shape[0]
    assert n % 512 == 0
    rest = a.shape[1:]
    return np.ascontiguousarray(
        a.reshape(n // 512, 128, 4, *rest).swapaxes(1, 2).reshape(a.shape)
    )


def _host_inputs(inputs):
    """Build the per-core in_maps from the full problem inputs."""
    g = {k: np.asarray(v) for k, v in inputs.items()}
    sz = np.array([float(g["sz_r"]), float(g["sz_c"])], dtype=np.float64)

    def f32(x):
        return np.ascontiguousarray(np.asarray(x, dtype=np.float32))

    def f16(x):
        return np.ascontiguousarray(np.asarray(x, dtype=np.float16))

    # Mmat [3, 40]: args rows of the scaled matmul (fp16: the 2^k/sz
    # coefficients and the integer idx values are exact in fp16; only the
    # pi/2 shift rounds, by ~5e-4 rad).
    M = np.zeros((3, 40), dtype=np.float64)
    for j in range(40):
        blk = j if j < 20 else j - 20
        k, cc = blk // 2, blk % 2
        M[cc, j] = (2.0**k) / sz[cc]
        M[2, j] = 0.0 if j < 20 else np.pi / 2
    mmat = f16(M)

    Wd1, bd1 = g["Wd1"], g["bd1"]
    Wd2, bd2 = g["Wd2"], g["bd2"]
    consts = {
        "mmat": mmat,
        "w1d": f32(Wd1[:, 0:1]),
        "b1d": f32(bd1.reshape(128, 1)),
        "w2td": f16(np.asarray(Wd2, np.float64).T),
        "b2bd": f32(np.tile(bd2.reshape(1, -1), (128, PC // 128))),
    }

    # posenc weights; X row order -> original pe column order
    sin_src = [2 + 4 * (i // 2) + (i % 2) for i in range(20)]
    cos_src = [2 + 4 * (i // 2) + 2 + (i % 2) for i in range(20)]
    cols = sin_src + cos_src
    for s, W1, b1, W2, b2, W3, b3, tv in (
        ("f", g["Wf1"], g["bf1"], g["Wf2"], g["bf2"], g["Wf3"], g["bf3"], (1.0, 0.0)),
        ("a", g["Wa1"], g["ba1"], g["Wa2"], g["ba2"], g["Wa3"], g["ba3"], (0.0, 1.0)),
    ):
        W1 = np.asarray(W1, np.float64)
        b1e = np.asarray(b1, np.float64) + W1[:, 42] * tv[0] + W1[:, 43] * tv[1]
        consts[f"w1a{s}"] = f16(W1[:, cols].T)               # [40, 64]
        consts[f"w1b{s}"] = f16((W1[:, 0:2] / sz[None, :]).T)  # [2, 64]
        consts[f"b1e{s}"] = f32(b1e.reshape(64, 1))
        consts[f"w2t{s}"] = f16(np.asarray(W2, np.float64).T)  # [64, 128]
        consts[f"b2c{s}"] = f32(np.asarray(b2).reshape(128, 1))
        consts[f"w3t{s}"] = f16(np.asarray(W3, np.float64).T)  # [128, 128]
        consts[f"b3b{s}"] = f32(np.tile(np.asarray(b3).reshape(1, -1), (128, PC // 128)))

    in_maps = []
    fi, gi = g["frontier_idx"], g["ghistory_idx"]
    pi_, ai = g["phistory_idx"], g["agent_pos"]
    dv = g["dist_vals"]
    for b in range(B):
        idxf = np.concatenate([np.asarray(fi[b]), np.asarray(gi[b])], axis=0)
        idxa_raw = np.concatenate([np.asarray(pi_[b]), np.asarray(ai[b])], axis=0)
        idxa = np.zeros((NAB, 2), dtype=np.float64)
        idxa[: idxa_raw.shape[0]] = np.asarray(idxa_raw, np.float64)

        idxf_p = _perm512(np.asarray(idxf, np.float64))
        idxa_p = _perm512(idxa)

        def with_ones(t, n):
            out = np.ones((3, n), dtype=np.float16)
            out[0:2, :] = t.T.astype(np.float16)
            return np.ascontiguousarray(out)

        m = dict(consts)
        m["idxf"] = with_ones(idxf_p, NFB)
        m["idxa"] = with_ones(idxa_p, NAB)
        dperm = _perm512(np.asarray(dv[b], np.float32).reshape(ND))
        m["dist"] = f32(dperm.reshape(128, 512))
        in_maps.append(m)
    return in_maps


def kernel(**inputs):
    global LAST_RESULT, _LEGALIZED
    nc = _get_program()
    if not _LEGALIZED:
        # CoreSim can't execute the injected EventSemaphores, so this runs
        # only on the hardware path.
        _legalize_waits(nc)
        _LEGALIZED = True
    in_maps = _host_inputs(inputs)
    trace = os.environ.get("BASS_TRACE", "") not in ("", "0")
    res = run_bass_kernel_spmd(nc, in_maps, core_ids=list(range(B)), trace=trace)
    LAST_RESULT = res
    r = res.results
    dist_enc = np.stack([r[b]["dist_out"] for b in range(B)]).astype(np.float32)
    frontier = np.stack([r[b]["front_out"] for b in range(B)]).astype(np.float32)
    agent = np.stack([r[b]["agent_out"] for b in range(B)]).astype(np.float32)
    phist = np.stack([r[b]["phist_out"] for b in range(B)]).astype(np.float32)
    ghist = np.stack([r[b]["ghist_out"] for b in range(B)]).astype(np.float32)
    return (dist_enc, frontier, agent, phist, ghist)


# revision 27
# speedup vs baseline: 1.3938x; 1.0082x over previous
"""Trainium2 Bass kernel for nn_Encoder_44238163149250.

Sharding: data-parallel over batch B=8 -> one batch element per NeuronCore.
Each core computes all five encoder outputs for its batch element.

Per-core structure (batch element b), dist and posenc chunks interleaved in
one Tile region so every engine stays loaded across the whole kernel:

  dist branch  (65536 pts, 512-pt chunks):
      y = relu(w1*log1p(d) + b1) @ W2.T + b2
      - log1p once on ACT over the whole [128,512] input; round-trip through
        a DRAM scratch so chunks re-load as [1,512] rows at base partition 0
        (SWDGE casts f32->fp16 during that reload)
      - broadcast across partitions with a K=1 ones outer-product matmul
      - h1 = Relu(scale*psum + bias) on ACT (per-partition scale/bias), fp16
      - layer 2 uses h1 slices as the matmul *stationary* operand
        (lhsT = h1[:,128j], rhs = W2.T fp16) so PSUM comes out in
        [points, feat] layout -> no transpose anywhere
      - bias-add + PSUM->SBUF on DVE; two chunks batched per output DMA
  posenc branches (frontier+ghist = fparams, phist+agent = aparams):
      - freq-scaled args via one K=3 fp16 matmul vs [idx_r; idx_c; 1]
        (integer idx and 2^k/512 coefficients are exact in fp16)
      - range-reduce to [-pi,pi]: k = round(x/2pi) via ACT Identity
        (scale+magic bias) + DVE subtract; r = x - k*fl32(2pi) via DVE
        scalar_tensor_tensor; clamp on DVE; cos rows are sin(x + pi/2)
        with the shift riding the ones row of the idx matmul
      - 3-layer fp16 MLP, layer biases folded into ACT per-partition bias,
        last layer via the same stationary-operand trick

  Point order inside each 512-pt chunk is host-permuted (position j*128+p
  holds point 4p+j) so each partition's DMA store is one contiguous 2KB run
  instead of 4 scattered 512B runs.
"""

import os

import numpy as np

import concourse.bass as bass
import concourse.bass_utils as _bass_utils
import concourse.mybir as mybir
import concourse.tile as tile
from concourse.bass_utils import run_bass_kernel_spmd

# The repo's walrus invocation passes --enable-ldw-opt=false, which leaves
# every matmul paying its LDWEIGHTS serially (~123ns each, ~100us across this
# kernel). Flip it on for our NEFF compiles.
_orig_run_command = _bass_utils.run_command


def _run_command_ldwopt(argv, **kwargs):
    argv = [
        a
        for a in argv
    ]
    return _orig_run_command(argv, **kwargs)


_bass_utils.run_command = _run_command_ldwopt

F32 = mybir.dt.float32
F16 = mybir.dt.float16
AF = mybir.ActivationFunctionType
ALU = mybir.AluOpType

NUM_FREQS = 10
B = 8

# per-core sizes
ND = 65536          # dist points (D*NF = 8*8192)
NFR = 8192          # frontier pts
NG = 2048           # ghist pts
NPH = 2048          # phist pts
NA = 8              # agent pts
NFB = NFR + NG      # F-branch pts  (fparams + type_frontier)
NAB = 2560          # A-branch pts padded (phist 2048 + agent 8 -> 2560)

PC = 512            # chunk size (points) for both branches
NDC = ND // PC      # 128 dist chunks
NFC = NFB // PC     # 20
NAC = NAB // PC     # 5

MAGIC = 12582912.0                       # 1.5 * 2**23: round-to-nearest trick
TWO_PI = 2.0 * np.pi
FL2PI = float(np.float32(TWO_PI))        # r = x - k*fl32(2pi): |err| <= 5e-5 rad
INV_2PI = 1.0 / TWO_PI
PI_LO = float(np.nextafter(np.float32(np.pi), np.float32(0)))  # f32 just below pi

_PROG = None        # cached nc build
_LEGALIZED = False  # wait-legalization applied to _PROG (HW path only)
LAST_RESULT = None  # BassKernelResults of the last kernel() call (for test.py)

DIST_ON = os.environ.get("K_DIST", "1") == "1"
POSENC_ON = os.environ.get("K_POSENC", "1") == "1"


def _legalize_waits(nc):
    """This toolchain's walrus accepts at most ONE sync wait per instruction
    (codegen raises 'Too many sync wait commands' otherwise), while Tile
    freely emits several. Hoist all but the last wait of each instruction
    onto same-engine EventSemaphore instructions inserted right before it —
    engines execute their queue in order, so sequential waits are equivalent
    to one multi-wait."""
    n = 0
    for f in nc.m.functions:
        for b in f.blocks:
            insts = list(b.instructions)
            out = []
            changed = False
            for i in insts:
                si = i.sync_info
                if si is not None and len(si.on_wait) > 1:
                    waits = list(si.on_wait)
                    for j, w in enumerate(waits[:-1]):
                        ev = mybir.InstEventSemaphore(
                            name=f"lw_{i.name}_{j}",
                            engine=i.engine,
                            ins=[],
                            outs=[],
                            sync_info=mybir.SyncInfo(on_wait=[w], on_update=[]),
                        )
                        out.append(ev)
                        n += 1
                    i.sync_info = mybir.SyncInfo(
                        on_wait=[waits[-1]], on_update=list(si.on_update)
                    )
                    changed = True
                out.append(i)
            if changed:
                cur = b.instructions
                del cur[:]
                for i in out:
                    cur.append(i)
    return n


def _build_program():
    nc = bass.Bass()

    # ---- DRAM I/O ----
    din = nc.dram_tensor("dist", [128, 512], F32, kind="ExternalInput")
    idx_f = nc.dram_tensor("idxf", [3, NFB], F16, kind="ExternalInput")
    idx_a = nc.dram_tensor("idxa", [3, NAB], F16, kind="ExternalInput")
    mmat = nc.dram_tensor("mmat", [3, 40], F16, kind="ExternalInput")
    w1d = nc.dram_tensor("w1d", [128, 1], F32, kind="ExternalInput")
    b1d = nc.dram_tensor("b1d", [128, 1], F32, kind="ExternalInput")
    w2td = nc.dram_tensor("w2td", [128, 128], F16, kind="ExternalInput")
    b2bd = nc.dram_tensor("b2bd", [128, PC], F32, kind="ExternalInput")
    brt = {}
    for s in ("f", "a"):
        brt[s] = dict(
            w1a=nc.dram_tensor(f"w1a{s}", [40, 64], F16, kind="ExternalInput"),
            w1b=nc.dram_tensor(f"w1b{s}", [2, 64], F16, kind="ExternalInput"),
            b1e=nc.dram_tensor(f"b1e{s}", [64, 1], F32, kind="ExternalInput"),
            w2t=nc.dram_tensor(f"w2t{s}", [64, 128], F16, kind="ExternalInput"),
            b2c=nc.dram_tensor(f"b2c{s}", [128, 1], F32, kind="ExternalInput"),
            w3t=nc.dram_tensor(f"w3t{s}", [128, 128], F16, kind="ExternalInput"),
            b3b=nc.dram_tensor(f"b3b{s}", [128, PC], F32, kind="ExternalInput"),
        )

    dout = nc.dram_tensor("dist_out", [ND, 128], F16, kind="ExternalOutput")
    fout = nc.dram_tensor("front_out", [NFR, 128], F16, kind="ExternalOutput")
    gout = nc.dram_tensor("ghist_out", [NG, 128], F16, kind="ExternalOutput")
    pout = nc.dram_tensor("phist_out", [NPH, 128], F16, kind="ExternalOutput")
    aout = nc.dram_tensor("agent_out", [NA, 128], F16, kind="ExternalOutput")

    # Views matching the in-chunk permutation: position j*128+p <-> point
    # 512c + 4p + j. Dist pairs two chunks per DMA.
    dview = dout.rearrange("(t h p g) o -> t p h g o", h=2, p=128, g=4)
    fview = fout.rearrange("(c p g) o -> c p g o", p=128, g=4)
    gview = gout.rearrange("(c p g) o -> c p g o", p=128, g=4)
    pview = pout.rearrange("(c p g) o -> c p g o", p=128, g=4)
    aview = aout.rearrange("(p j) o -> p j o", j=4)

    with tile.TileContext(nc) as tc:
        with tc.tile_pool(name="const", bufs=1) as cp:
            # dist input + log1p first so the scratch/ct chain starts before
            # the (many) constant loads occupy the queues
            dist_sb = cp.tile([128, 512], F32, tag="dist_sb")
            nc.sync.dma_start(out=dist_sb[:], in_=din[:, :])
            logd = cp.tile([128, 512], F32, tag="logd")
            nc.scalar.activation(logd[:], dist_sb[:], AF.Ln, bias=1.0, scale=1.0)

            def cload(dram, shape, tag, dt=F32):
                t = cp.tile(shape, dt, tag=tag)
                nc.sync.dma_start(out=t[:], in_=dram[:, :])
                return t

            w1d_t = cload(w1d, [128, 1], "w1d")
            b1d_t = cload(b1d, [128, 1], "b1d")
            w2td_t = cload(w2td, [128, 128], "w2td", F16)
            b2bd_t = cload(b2bd, [128, PC], "b2bd")
            mmat_t = cload(mmat, [3, 40], "mmat", F16)
            brc = {}
            for s in ("f", "a"):
                d = brt[s]
                brc[s] = dict(
                    w1a=cload(d["w1a"], [40, 64], f"w1a{s}", F16),
                    w1b=cload(d["w1b"], [2, 64], f"w1b{s}", F16),
                    b1e=cload(d["b1e"], [64, 1], f"b1e{s}"),
                    w2t=cload(d["w2t"], [64, 128], f"w2t{s}", F16),
                    b2c=cload(d["b2c"], [128, 1], f"b2c{s}"),
                    w3t=cload(d["w3t"], [128, 128], f"w3t{s}", F16),
                    b3b=cload(d["b3b"], [128, PC], f"b3b{s}"),
                )
            idxf_t = cload(idx_f, [3, NFB], "idxf", F16)
            idxa_t = cload(idx_a, [3, NAB], "idxa", F16)

            ones_l = cp.tile([1, 128], F16, tag="ones")
            nc.vector.memset(ones_l[:], 1.0)
            magic_c = cp.tile([40, 1], F32, tag="magic")
            nc.vector.memset(magic_c[:], MAGIC)

            with (
                tc.tile_pool(name="ddram", bufs=1, space="DRAM") as ddram,
                tc.tile_pool(name="dsb", bufs=4) as dsb,
                tc.tile_pool(name="esb", bufs=4) as esb,
                tc.tile_pool(name="dpb", bufs=2, space="PSUM") as dpb,
                tc.tile_pool(name="dpo", bufs=2, space="PSUM") as dpo,
                tc.tile_pool(name="eps", bufs=1, space="PSUM") as eps,
                tc.tile_pool(name="ep1", bufs=1, space="PSUM") as ep1,
                tc.tile_pool(name="ep2", bufs=1, space="PSUM") as ep2,
                tc.tile_pool(name="epo", bufs=1, space="PSUM") as epo,
            ):
                scr = ddram.tile([128, 512], F32, tag="scr")
                nc.gpsimd.dma_start(out=scr[:], in_=logd[:])

                def dist_chunk(c, ot_half):
                    ct = dsb.tile([1, PC], F16, tag="ct")
                    nc.gpsimd.dma_start(out=ct[:], in_=scr[c : c + 1, :])
                    psb = dpb.tile([128, PC], F32, tag="psb")
                    nc.tensor.matmul(
                        psb[:], lhsT=ones_l[:], rhs=ct[0:1, :],
                        start=True, stop=True,
                    )
                    h1 = dsb.tile([128, PC], F16, tag="h1")
                    nc.scalar.activation(
                        h1[:], psb[:], AF.Relu, bias=b1d_t[:], scale=w1d_t[:]
                    )
                    pso = dpo.tile([128, PC], F32, tag="pso")
                    for j in range(PC // 128):
                        nc.tensor.matmul(
                            pso[:, j * 128 : (j + 1) * 128],
                            lhsT=h1[:, j * 128 : (j + 1) * 128],
                            rhs=w2td_t[:],
                            start=True,
                            stop=True,
                        )
                    nc.vector.tensor_add(ot_half, pso[:], b2bd_t[:])

                def posenc_chunk(s, idx_t, c):
                    w = brc[s]
                    lo = c * PC
                    pss = eps.tile([40, PC], F32, tag="pss")
                    nc.tensor.matmul(
                        pss[:], lhsT=mmat_t[:], rhs=idx_t[:, lo : lo + PC],
                        start=True, stop=True,
                    )
                    # k = round(args/2pi): scale+magic on ACT, subtract on DVE
                    kt1 = esb.tile([40, PC], F32, tag="kt1")
                    nc.scalar.activation(
                        kt1[:], pss[:], AF.Identity,
                        bias=magic_c[:], scale=INV_2PI,
                    )
                    kt = esb.tile([40, PC], F32, tag="kt")
                    nc.vector.tensor_scalar(
                        out=kt[:], in0=kt1[:], scalar1=MAGIC, scalar2=None,
                        op0=ALU.subtract,
                    )
                    rt = esb.tile([40, PC], F32, tag="rt")
                    nc.vector.scalar_tensor_tensor(
                        out=rt[:], in0=kt[:], scalar=-FL2PI,
                        in1=pss[:], op0=ALU.mult, op1=ALU.add,
                    )
                    r3 = esb.tile([40, PC], F32, tag="r3")
                    nc.vector.tensor_scalar(
                        out=r3[:], in0=rt[:],
                        scalar1=PI_LO, scalar2=-PI_LO,
                        op0=ALU.min, op1=ALU.max,
                    )
                    xs = esb.tile([40, PC], F16, tag="xs")
                    nc.scalar.activation(xs[:], r3[:], AF.Sin)

                    ps1 = ep1.tile([64, PC], F32, tag="ps1")
                    nc.tensor.matmul(
                        ps1[:], lhsT=w["w1a"][:], rhs=xs[:],
                        start=True, stop=False,
                    )
                    nc.tensor.matmul(
                        ps1[:], lhsT=w["w1b"][:], rhs=idx_t[0:2, lo : lo + PC],
                        start=False, stop=True,
                    )
                    h1e = esb.tile([64, PC], F16, tag="h1e")
                    nc.scalar.activation(h1e[:], ps1[:], AF.Relu, bias=w["b1e"][:])

                    ps2 = ep2.tile([128, PC], F32, tag="ps2")
                    nc.tensor.matmul(
                        ps2[:], lhsT=w["w2t"][:], rhs=h1e[:], start=True, stop=True
                    )
                    h2e = esb.tile([128, PC], F16, tag="h2e")
                    nc.scalar.activation(h2e[:], ps2[:], AF.Relu, bias=w["b2c"][:])

                    po = epo.tile([128, PC], F32, tag="po")
                    for j in range(PC // 128):
                        nc.tensor.matmul(
                            po[:, j * 128 : (j + 1) * 128],
                            lhsT=h2e[:, j * 128 : (j + 1) * 128],
                            rhs=w["w3t"][:],
                            start=True,
                            stop=True,
                        )
                    oute = esb.tile([128, PC], F16, tag="oute")
                    nc.vector.tensor_add(oute[:], po[:], w["b3b"][:])

                    src = oute[:].rearrange("p (g o) -> p g o", g=4)
                    if s == "f":
                        if c < NFR // PC:
                            nc.sync.dma_start(out=fview[c, :, :, :], in_=src)
                        else:
                            nc.sync.dma_start(
                                out=gview[c - NFR // PC, :, :, :], in_=src
                            )
                    else:
                        if c < NPH // PC:
                            nc.sync.dma_start(out=pview[c, :, :, :], in_=src)
                        else:
                            nc.sync.dma_start(
                                out=aview[:, :, :],
                                in_=oute[0:2, :].rearrange("p (j o) -> p j o", j=4),
                            )

                pe_jobs = []
                if POSENC_ON:
                    pe_jobs = [("f", idxf_t, c) for c in range(NFC)] + [
                        ("a", idxa_t, c) for c in range(NAC)
                    ]
                n_pairs = NDC // 2 if DIST_ON else 0
                emitted = 0
                # posenc has a much shorter input lead-in than dist (which
                # waits on the log1p -> DRAM scratch round-trip); start with a
                # few posenc chunks so the PE isn't idle at kernel start
                for _ in range(min(5, len(pe_jobs))):
                    posenc_chunk(*pe_jobs[emitted])
                    emitted += 1
                for t in range(n_pairs):
                    ot = dsb.tile([128, 2 * PC], F16, tag="ot")
                    for h in range(2):
                        dist_chunk(2 * t + h, ot[:, h * PC : (h + 1) * PC])
                    nc.sync.dma_start(
                        out=dview[t, :, :, :, :],
                        in_=ot[:].rearrange("p (h g o) -> p h g o", h=2, g=4),
                    )
                    want = (t + 1) * len(pe_jobs) // n_pairs
                    while emitted < want:
                        posenc_chunk(*pe_jobs[emitted])
                        emitted += 1
                while emitted < len(pe_jobs):
                    posenc_chunk(*pe_jobs[emitted])
                    emitted += 1
    return nc


def _get_program():
    global _PROG
    if _PROG is None:
        _PROG = _build_program()
    return _PROG


def _perm512(a):
    """Permute rows within each 512 block: new position j*128+p <- old 4p+j."""
    n = a.shape[0]
    assert n % 512 == 0
    rest = a.shape[1:]
    return np.ascontiguousarray(
        a.reshape(n // 512, 128, 4, *rest).swapaxes(1, 2).reshape(a.shape)
    )


def _host_inputs(inputs):
    """Build the per-core in_maps from the full problem inputs."""
    g = {k: np.asarray(v) for k, v in inputs.items()}
    sz = np.array([float(g["sz_r"]), float(g["sz_c"])], dtype=np.float64)

    def f32(x):
        return np.ascontiguousarray(np.asarray(x, dtype=np.float32))

    def f16(x):
        return np.ascontiguousarray(np.asarray(x, dtype=np.float16))

    # Mmat [3, 40]: args rows of the scaled matmul (fp16: the 2^k/sz
    # coefficients and the integer idx values are exact in fp16; only the
    # pi/2 shift rounds, by ~5e-4 rad).
    M = np.zeros((3, 40), dtype=np.float64)
    for j in range(40):
        blk = j if j < 20 else j - 20
        k, cc = blk // 2, blk % 2
        M[cc, j] = (2.0**k) / sz[cc]
        M[2, j] = 0.0 if j < 20 else np.pi / 2
    mmat = f16(M)

    Wd1, bd1 = g["Wd1"], g["bd1"]
    Wd2, bd2 = g["Wd2"], g["bd2"]
    consts = {
        "mmat": mmat,
        "w1d": f32(Wd1[:, 0:1]),
        "b1d": f32(bd1.reshape(128, 1)),
        "w2td": f16(np.asarray(Wd2, np.float64).T),
        "b2bd": f32(np.tile(bd2.reshape(1, -1), (128, PC // 128))),
    }

    # posenc weights; X row order -> original pe column order
    sin_src = [2 + 4 * (i // 2) + (i % 2) for i in range(20)]
    cos_src = [2 + 4 * (i // 2) + 2 + (i % 2) for i in range(20)]
    cols = sin_src + cos_src
    for s, W1, b1, W2, b2, W3, b3, tv in (
        ("f", g["Wf1"], g["bf1"], g["Wf2"], g["bf2"], g["Wf3"], g["bf3"], (1.0, 0.0)),
        ("a", g["Wa1"], g["ba1"], g["Wa2"], g["ba2"], g["Wa3"], g["ba3"], (0.0, 1.0)),
    ):
        W1 = np.asarray(W1, np.float64)
        b1e = np.asarray(b1, np.float64) + W1[:, 42] * tv[0] + W1[:, 43] * tv[1]
        consts[f"w1a{s}"] = f16(W1[:, cols].T)               # [40, 64]
        consts[f"w1b{s}"] = f16((W1[:, 0:2] / sz[None, :]).T)  # [2, 64]
        consts[f"b1e{s}"] = f32(b1e.reshape(64, 1))
        consts[f"w2t{s}"] = f16(np.asarray(W2, np.float64).T)  # [64, 128]
        consts[f"b2c{s}"] = f32(np.asarray(b2).reshape(128, 1))
        consts[f"w3t{s}"] = f16(np.asarray(W3, np.float64).T)  # [128, 128]
        consts[f"b3b{s}"] = f32(np.tile(np.asarray(b3).reshape(1, -1), (128, PC // 128)))

    in_maps = []
    fi, gi = g["frontier_idx"], g["ghistory_idx"]
    pi_, ai = g["phistory_idx"], g["agent_pos"]
    dv = g["dist_vals"]
    for b in range(B):
        idxf = np.concatenate([np.asarray(fi[b]), np.asarray(gi[b])], axis=0)
        idxa_raw = np.concatenate([np.asarray(pi_[b]), np.asarray(ai[b])], axis=0)
        idxa = np.zeros((NAB, 2), dtype=np.float64)
        idxa[: idxa_raw.shape[0]] = np.asarray(idxa_raw, np.float64)

        idxf_p = _perm512(np.asarray(idxf, np.float64))
        idxa_p = _perm512(idxa)

        def with_ones(t, n):
            out = np.ones((3, n), dtype=np.float16)
            out[0:2, :] = t.T.astype(np.float16)
            return np.ascontiguousarray(out)

        m = dict(consts)
        m["idxf"] = with_ones(idxf_p, NFB)
        m["idxa"] = with_ones(idxa_p, NAB)
        dperm = _perm512(np.asarray(dv[b], np.float32).reshape(ND))
        m["dist"] = f32(dperm.reshape(128, 512))
        in_maps.append(m)
    return in_maps


def kernel(**inputs):
    global LAST_RESULT, _LEGALIZED
    nc = _get_program()
    if not _LEGALIZED:
        # CoreSim can't execute the injected EventSemaphores, so this runs
        # only on the hardware path.
        _legalize_waits(nc)
        _LEGALIZED = True
    in_maps = _host_inputs(inputs)
    trace = os.environ.get("BASS_TRACE", "") not in ("", "0")
    res = run_bass_kernel_spmd(nc, in_maps, core_ids=list(range(B)), trace=trace)
    LAST_RESULT = res
    r = res.results
    dist_enc = np.stack([r[b]["dist_out"] for b in range(B)]).astype(np.float32)
    frontier = np.stack([r[b]["front_out"] for b in range(B)]).astype(np.float32)
    agent = np.stack([r[b]["agent_out"] for b in range(B)]).astype(np.float32)
    phist = np.stack([r[b]["phist_out"] for b in range(B)]).astype(np.float32)
    ghist = np.stack([r[b]["ghist_out"] for b in range(B)]).astype(np.float32)
    return (dist_enc, frontier, agent, phist, ghist)


# revision 29
# speedup vs baseline: 1.4008x; 1.0050x over previous
"""Trainium2 Bass kernel for nn_Encoder_44238163149250.

Sharding: data-parallel over batch B=8 -> one batch element per NeuronCore.
Each core computes all five encoder outputs for its batch element.

Per-core structure (batch element b), dist and posenc chunks interleaved in
one Tile region so every engine stays loaded across the whole kernel:

  dist branch  (65536 pts, 512-pt chunks):
      y = relu(w1*log1p(d) + b1) @ W2.T + b2
      - log1p once on ACT over the whole [128,512] input; round-trip through
        a DRAM scratch so chunks re-load as [1,512] rows at base partition 0
        (SWDGE casts f32->fp16 during that reload)
      - broadcast across partitions with a K=1 ones outer-product matmul
      - h1 = Relu(scale*psum + bias) on ACT (per-partition scale/bias), fp16
      - layer 2 uses h1 slices as the matmul *stationary* operand
        (lhsT = h1[:,128j], rhs = W2.T fp16) so PSUM comes out in
        [points, feat] layout -> no transpose anywhere
      - bias-add + PSUM->SBUF on DVE; two chunks batched per output DMA
  posenc branches (frontier+ghist = fparams, phist+agent = aparams):
      - freq-scaled args via one K=3 fp16 matmul vs [idx_r; idx_c; 1]
        (integer idx and 2^k/512 coefficients are exact in fp16)
      - range-reduce to [-pi,pi]: k = round(x/2pi) via ACT Identity
        (scale+magic bias) + DVE subtract; r = x - k*fl32(2pi) via DVE
        scalar_tensor_tensor; clamp on DVE; cos rows are sin(x + pi/2)
        with the shift riding the ones row of the idx matmul
      - 3-layer fp16 MLP, layer biases folded into ACT per-partition bias,
        last layer via the same stationary-operand trick

  Point order inside each 512-pt chunk is host-permuted (position j*128+p
  holds point 4p+j) so each partition's DMA store is one contiguous 2KB run
  instead of 4 scattered 512B runs.
"""

import os

import numpy as np

import concourse.bass as bass
import concourse.mybir as mybir
import concourse.tile as tile
from concourse.bass_utils import run_bass_kernel_spmd

F32 = mybir.dt.float32
F16 = mybir.dt.float16
AF = mybir.ActivationFunctionType
ALU = mybir.AluOpType

NUM_FREQS = 10
B = 8

# per-core sizes
ND = 65536          # dist points (D*NF = 8*8192)
NFR = 8192          # frontier pts
NG = 2048           # ghist pts
NPH = 2048          # phist pts
NA = 8              # agent pts
NFB = NFR + NG      # F-branch pts  (fparams + type_frontier)
NAB = 2560          # A-branch pts padded (phist 2048 + agent 8 -> 2560)

PC = 512            # chunk size (points) for both branches
NDC = ND // PC      # 128 dist chunks
NFC = NFB // PC     # 20
NAC = NAB // PC     # 5

MAGIC = 12582912.0                       # 1.5 * 2**23: round-to-nearest trick
TWO_PI = 2.0 * np.pi
FL2PI = float(np.float32(TWO_PI))        # r = x - k*fl32(2pi): |err| <= 5e-5 rad
INV_2PI = 1.0 / TWO_PI
PI_LO = float(np.nextafter(np.float32(np.pi), np.float32(0)))  # f32 just below pi

_PROG = None        # cached nc build
_LEGALIZED = False  # wait-legalization applied to _PROG (HW path only)
LAST_RESULT = None  # BassKernelResults of the last kernel() call (for test.py)

DIST_ON = os.environ.get("K_DIST", "1") == "1"
POSENC_ON = os.environ.get("K_POSENC", "1") == "1"


def _legalize_waits(nc):
    """This toolchain's walrus accepts at most ONE sync wait per instruction
    (codegen raises 'Too many sync wait commands' otherwise), while Tile
    freely emits several. Hoist all but the last wait of each instruction
    onto same-engine EventSemaphore instructions inserted right before it —
    engines execute their queue in order, so sequential waits are equivalent
    to one multi-wait."""
    n = 0
    for f in nc.m.functions:
        for b in f.blocks:
            insts = list(b.instructions)
            out = []
            changed = False
            for i in insts:
                si = i.sync_info
                if si is not None and len(si.on_wait) > 1:
                    waits = list(si.on_wait)
                    for j, w in enumerate(waits[:-1]):
                        ev = mybir.InstEventSemaphore(
                            name=f"lw_{i.name}_{j}",
                            engine=i.engine,
                            ins=[],
                            outs=[],
                            sync_info=mybir.SyncInfo(on_wait=[w], on_update=[]),
                        )
                        out.append(ev)
                        n += 1
                    i.sync_info = mybir.SyncInfo(
                        on_wait=[waits[-1]], on_update=list(si.on_update)
                    )
                    changed = True
                out.append(i)
            if changed:
                cur = b.instructions
                del cur[:]
                for i in out:
                    cur.append(i)
    return n


def _build_program():
    nc = bass.Bass()

    # ---- DRAM I/O ----
    din = nc.dram_tensor("dist", [128, 512], F32, kind="ExternalInput")
    idx_f = nc.dram_tensor("idxf", [3, NFB], F16, kind="ExternalInput")
    idx_a = nc.dram_tensor("idxa", [3, NAB], F16, kind="ExternalInput")
    mmat = nc.dram_tensor("mmat", [3, 40], F16, kind="ExternalInput")
    w1d = nc.dram_tensor("w1d", [128, 1], F32, kind="ExternalInput")
    b1d = nc.dram_tensor("b1d", [128, 1], F32, kind="ExternalInput")
    w2td = nc.dram_tensor("w2td", [128, 128], F16, kind="ExternalInput")
    b2bd = nc.dram_tensor("b2bd", [128, PC], F32, kind="ExternalInput")
    brt = {}
    for s in ("f", "a"):
        brt[s] = dict(
            w1a=nc.dram_tensor(f"w1a{s}", [40, 64], F16, kind="ExternalInput"),
            w1b=nc.dram_tensor(f"w1b{s}", [2, 64], F16, kind="ExternalInput"),
            b1e=nc.dram_tensor(f"b1e{s}", [64, 1], F32, kind="ExternalInput"),
            w2t=nc.dram_tensor(f"w2t{s}", [64, 128], F16, kind="ExternalInput"),
            b2c=nc.dram_tensor(f"b2c{s}", [128, 1], F32, kind="ExternalInput"),
            w3t=nc.dram_tensor(f"w3t{s}", [128, 128], F16, kind="ExternalInput"),
            b3b=nc.dram_tensor(f"b3b{s}", [128, PC], F32, kind="ExternalInput"),
        )

    dout = nc.dram_tensor("dist_out", [ND, 128], F16, kind="ExternalOutput")
    fout = nc.dram_tensor("front_out", [NFR, 128], F16, kind="ExternalOutput")
    gout = nc.dram_tensor("ghist_out", [NG, 128], F16, kind="ExternalOutput")
    pout = nc.dram_tensor("phist_out", [NPH, 128], F16, kind="ExternalOutput")
    aout = nc.dram_tensor("agent_out", [NA, 128], F16, kind="ExternalOutput")

    # Views matching the in-chunk permutation: position j*128+p <-> point
    # 512c + 4p + j. Dist pairs two chunks per DMA.
    dview = dout.rearrange("(t h p g) o -> t p h g o", h=2, p=128, g=4)
    fview = fout.rearrange("(c p g) o -> c p g o", p=128, g=4)
    gview = gout.rearrange("(c p g) o -> c p g o", p=128, g=4)
    pview = pout.rearrange("(c p g) o -> c p g o", p=128, g=4)
    aview = aout.rearrange("(p j) o -> p j o", j=4)

    with tile.TileContext(nc) as tc:
        with tc.tile_pool(name="const", bufs=1) as cp:
            # dist input + log1p first so the scratch/ct chain starts before
            # the (many) constant loads occupy the queues
            dist_sb = cp.tile([128, 512], F32, tag="dist_sb")
            nc.sync.dma_start(out=dist_sb[:], in_=din[:, :])
            logd = cp.tile([128, 512], F32, tag="logd")
            nc.scalar.activation(logd[:], dist_sb[:], AF.Ln, bias=1.0, scale=1.0)

            def cload(dram, shape, tag, dt=F32):
                t = cp.tile(shape, dt, tag=tag)
                nc.sync.dma_start(out=t[:], in_=dram[:, :])
                return t

            w1d_t = cload(w1d, [128, 1], "w1d")
            b1d_t = cload(b1d, [128, 1], "b1d")
            w2td_t = cload(w2td, [128, 128], "w2td", F16)
            b2bd_t = cload(b2bd, [128, PC], "b2bd")
            mmat_t = cload(mmat, [3, 40], "mmat", F16)
            brc = {}
            for s in ("f", "a"):
                d = brt[s]
                brc[s] = dict(
                    w1a=cload(d["w1a"], [40, 64], f"w1a{s}", F16),
                    w1b=cload(d["w1b"], [2, 64], f"w1b{s}", F16),
                    b1e=cload(d["b1e"], [64, 1], f"b1e{s}"),
                    w2t=cload(d["w2t"], [64, 128], f"w2t{s}", F16),
                    b2c=cload(d["b2c"], [128, 1], f"b2c{s}"),
                    w3t=cload(d["w3t"], [128, 128], f"w3t{s}", F16),
                    b3b=cload(d["b3b"], [128, PC], f"b3b{s}"),
                )
            idxf_t = cload(idx_f, [3, NFB], "idxf", F16)
            idxa_t = cload(idx_a, [3, NAB], "idxa", F16)

            ones_l = cp.tile([1, 128], F16, tag="ones")
            nc.vector.memset(ones_l[:], 1.0)
            magic_c = cp.tile([40, 1], F32, tag="magic")
            nc.vector.memset(magic_c[:], MAGIC)

            with (
                tc.tile_pool(name="ddram", bufs=1, space="DRAM") as ddram,
                tc.tile_pool(name="dsb", bufs=4) as dsb,
                tc.tile_pool(name="esb", bufs=4) as esb,
                tc.tile_pool(name="dpb", bufs=2, space="PSUM") as dpb,
                tc.tile_pool(name="dpo", bufs=2, space="PSUM") as dpo,
                tc.tile_pool(name="eps", bufs=1, space="PSUM") as eps,
                tc.tile_pool(name="ep1", bufs=1, space="PSUM") as ep1,
                tc.tile_pool(name="ep2", bufs=1, space="PSUM") as ep2,
                tc.tile_pool(name="epo", bufs=1, space="PSUM") as epo,
            ):
                scr = ddram.tile([128, 512], F32, tag="scr")
                nc.gpsimd.dma_start(out=scr[:], in_=logd[:])

                def dist_chunk(c, ot_half):
                    ct = dsb.tile([1, PC], F16, tag="ct")
                    nc.gpsimd.dma_start(out=ct[:], in_=scr[c : c + 1, :])
                    psb = dpb.tile([128, PC], F32, tag="psb")
                    nc.tensor.matmul(
                        psb[:], lhsT=ones_l[:], rhs=ct[0:1, :],
                        start=True, stop=True,
                    )
                    h1 = dsb.tile([128, PC], F16, tag="h1")
                    nc.scalar.activation(
                        h1[:], psb[:], AF.Relu, bias=b1d_t[:], scale=w1d_t[:]
                    )
                    pso = dpo.tile([128, PC], F32, tag="pso")
                    for j in range(PC // 128):
                        nc.tensor.matmul(
                            pso[:, j * 128 : (j + 1) * 128],
                            lhsT=h1[:, j * 128 : (j + 1) * 128],
                            rhs=w2td_t[:],
                            start=True,
                            stop=True,
                        )
                    nc.vector.tensor_add(ot_half, pso[:], b2bd_t[:])

                def posenc_chunk(s, idx_t, c):
                    w = brc[s]
                    lo = c * PC
                    pss = eps.tile([40, PC], F32, tag="pss")
                    nc.tensor.matmul(
                        pss[:], lhsT=mmat_t[:], rhs=idx_t[:, lo : lo + PC],
                        start=True, stop=True,
                    )
                    # k = round(args/2pi): scale+magic on ACT, subtract on DVE
                    kt1 = esb.tile([40, PC], F32, tag="kt1")
                    nc.scalar.activation(
                        kt1[:], pss[:], AF.Identity,
                        bias=magic_c[:], scale=INV_2PI,
                    )
                    kt = esb.tile([40, PC], F32, tag="kt")
                    nc.vector.tensor_scalar(
                        out=kt[:], in0=kt1[:], scalar1=MAGIC, scalar2=None,
                        op0=ALU.subtract,
                    )
                    rt = esb.tile([40, PC], F32, tag="rt")
                    nc.vector.scalar_tensor_tensor(
                        out=rt[:], in0=kt[:], scalar=-FL2PI,
                        in1=pss[:], op0=ALU.mult, op1=ALU.add,
                    )
                    r3 = esb.tile([40, PC], F32, tag="r3")
                    nc.vector.tensor_scalar(
                        out=r3[:], in0=rt[:],
                        scalar1=PI_LO, scalar2=-PI_LO,
                        op0=ALU.min, op1=ALU.max,
                    )
                    xs = esb.tile([40, PC], F16, tag="xs")
                    nc.scalar.activation(xs[:], r3[:], AF.Sin)

                    ps1 = ep1.tile([64, PC], F32, tag="ps1")
                    nc.tensor.matmul(
                        ps1[:], lhsT=w["w1a"][:], rhs=xs[:],
                        start=True, stop=False,
                    )
                    nc.tensor.matmul(
                        ps1[:], lhsT=w["w1b"][:], rhs=idx_t[0:2, lo : lo + PC],
                        start=False, stop=True,
                    )
                    h1e = esb.tile([64, PC], F16, tag="h1e")
                    nc.scalar.activation(h1e[:], ps1[:], AF.Relu, bias=w["b1e"][:])

                    ps2 = ep2.tile([128, PC], F32, tag="ps2")
                    nc.tensor.matmul(
                        ps2[:], lhsT=w["w2t"][:], rhs=h1e[:], start=True, stop=True
                    )
                    h2e = esb.tile([128, PC], F16, tag="h2e")
                    nc.scalar.activation(h2e[:], ps2[:], AF.Relu, bias=w["b2c"][:])

                    po = epo.tile([128, PC], F32, tag="po")
                    for j in range(PC // 128):
                        nc.tensor.matmul(
                            po[:, j * 128 : (j + 1) * 128],
                            lhsT=h2e[:, j * 128 : (j + 1) * 128],
                            rhs=w["w3t"][:],
                            start=True,
                            stop=True,
                        )
                    oute = esb.tile([128, PC], F16, tag="oute")
                    nc.vector.tensor_add(oute[:], po[:], w["b3b"][:])

                    src = oute[:].rearrange("p (g o) -> p g o", g=4)
                    if s == "f":
                        if c < NFR // PC:
                            nc.sync.dma_start(out=fview[c, :, :, :], in_=src)
                        else:
                            nc.sync.dma_start(
                                out=gview[c - NFR // PC, :, :, :], in_=src
                            )
                    else:
                        if c < NPH // PC:
                            nc.sync.dma_start(out=pview[c, :, :, :], in_=src)
                        else:
                            nc.sync.dma_start(
                                out=aview[:, :, :],
                                in_=oute[0:2, :].rearrange("p (j o) -> p j o", j=4),
                            )

                pe_jobs = []
                if POSENC_ON:
                    pe_jobs = [("f", idxf_t, c) for c in range(NFC)] + [
                        ("a", idxa_t, c) for c in range(NAC)
                    ]
                n_pairs = NDC // 2 if DIST_ON else 0
                emitted = 0
                # posenc has a much shorter input lead-in than dist (which
                # waits on the log1p -> DRAM scratch round-trip); start with a
                # few posenc chunks so the PE isn't idle at kernel start
                for _ in range(min(5, len(pe_jobs))):
                    posenc_chunk(*pe_jobs[emitted])
                    emitted += 1
                for t in range(n_pairs):
                    ot = dsb.tile([128, 2 * PC], F16, tag="ot")
                    for h in range(2):
                        dist_chunk(2 * t + h, ot[:, h * PC : (h + 1) * PC])
                    nc.sync.dma_start(
                        out=dview[t, :, :, :, :],
                        in_=ot[:].rearrange("p (h g o) -> p h g o", h=2, g=4),
                    )
                    want = (t + 1) * len(pe_jobs) // n_pairs
                    while emitted < want:
                        posenc_chunk(*pe_jobs[emitted])
                        emitted += 1
                while emitted < len(pe_jobs):
                    posenc_chunk(*pe_jobs[emitted])
                    emitted += 1
    return nc


def _get_program():
    global _PROG
    if _PROG is None:
        _PROG = _build_program()
    return _PROG


def _perm512(a):
    """Permute rows within each 512 block: new position j*128+p <- old 4p+j."""
    n = a.shape[0]
    assert n % 512 == 0
    rest = a.shape[1:]
    return np.ascontiguousarray(
        a.reshape(n // 512, 128, 4, *rest).swapaxes(1, 2).reshape(a.shape)
    )


def _host_inputs(inputs):
    """Build the per-core in_maps from the full problem inputs."""
    g = {k: np.asarray(v) for k, v in inputs.items()}
    sz = np.array([float(g["sz_r"]), float(g["sz_c"])], dtype=np.float64)

    def f32(x):
        return np.ascontiguousarray(np.asarray(x, dtype=np.float32))

    def f16(x):
        return np.ascontiguousarray(np.asarray(x, dtype=np.float16))

    # Mmat [3, 40]: args rows of the scaled matmul (fp16: the 2^k/sz
    # coefficients and the integer idx values are exact in fp16; only the
    # pi/2 shift rounds, by ~5e-4 rad).
    M = np.zeros((3, 40), dtype=np.float64)
    for j in range(40):
        blk = j if j < 20 else j - 20
        k, cc = blk // 2, blk % 2
        M[cc, j] = (2.0**k) / sz[cc]
        M[2, j] = 0.0 if j < 20 else np.pi / 2
    mmat = f16(M)

    Wd1, bd1 = g["Wd1"], g["bd1"]
    Wd2, bd2 = g["Wd2"], g["bd2"]
    consts = {
        "mmat": mmat,
        "w1d": f32(Wd1[:, 0:1]),
        "b1d": f32(bd1.reshape(128, 1)),
        "w2td": f16(np.asarray(Wd2, np.float64).T),
        "b2bd": f32(np.tile(bd2.reshape(1, -1), (128, PC // 128))),
    }

    # posenc weights; X row order -> original pe column order
    sin_src = [2 + 4 * (i // 2) + (i % 2) for i in range(20)]
    cos_src = [2 + 4 * (i // 2) + 2 + (i % 2) for i in range(20)]
    cols = sin_src + cos_src
    for s, W1, b1, W2, b2, W3, b3, tv in (
        ("f", g["Wf1"], g["bf1"], g["Wf2"], g["bf2"], g["Wf3"], g["bf3"], (1.0, 0.0)),
        ("a", g["Wa1"], g["ba1"], g["Wa2"], g["ba2"], g["Wa3"], g["ba3"], (0.0, 1.0)),
    ):
        W1 = np.asarray(W1, np.float64)
        b1e = np.asarray(b1, np.float64) + W1[:, 42] * tv[0] + W1[:, 43] * tv[1]
        consts[f"w1a{s}"] = f16(W1[:, cols].T)               # [40, 64]
        consts[f"w1b{s}"] = f16((W1[:, 0:2] / sz[None, :]).T)  # [2, 64]
        consts[f"b1e{s}"] = f32(b1e.reshape(64, 1))
        consts[f"w2t{s}"] = f16(np.asarray(W2, np.float64).T)  # [64, 128]
        consts[f"b2c{s}"] = f32(np.asarray(b2).reshape(128, 1))
        consts[f"w3t{s}"] = f16(np.asarray(W3, np.float64).T)  # [128, 128]
        consts[f"b3b{s}"] = f32(np.tile(np.asarray(b3).reshape(1, -1), (128, PC // 128)))

    in_maps = []
    fi, gi = g["frontier_idx"], g["ghistory_idx"]
    pi_, ai = g["phistory_idx"], g["agent_pos"]
    dv = g["dist_vals"]
    for b in range(B):
        idxf = np.concatenate([np.asarray(fi[b]), np.asarray(gi[b])], axis=0)
        idxa_raw = np.concatenate([np.asarray(pi_[b]), np.asarray(ai[b])], axis=0)
        idxa = np.zeros((NAB, 2), dtype=np.float64)
        idxa[: idxa_raw.shape[0]] = np.asarray(idxa_raw, np.float64)

        idxf_p = _perm512(np.asarray(idxf, np.float64))
        idxa_p = _perm512(idxa)

        def with_ones(t, n):
            out = np.ones((3, n), dtype=np.float16)
            out[0:2, :] = t.T.astype(np.float16)
            return np.ascontiguousarray(out)

        m = dict(consts)
        m["idxf"] = with_ones(idxf_p, NFB)
        m["idxa"] = with_ones(idxa_p, NAB)
        dperm = _perm512(np.asarray(dv[b], np.float32).reshape(ND))
        m["dist"] = f32(dperm.reshape(128, 512))
        in_maps.append(m)
    return in_maps


def kernel(**inputs):
    global LAST_RESULT, _LEGALIZED
    nc = _get_program()
    if not _LEGALIZED:
        # CoreSim can't execute the injected EventSemaphores, so this runs
        # only on the hardware path.
        _legalize_waits(nc)
        _LEGALIZED = True
    in_maps = _host_inputs(inputs)
    trace = os.environ.get("BASS_TRACE", "") not in ("", "0")
    if trace:
        try:
            from antenv.axon_hooks import get_axon_ntff_profile_hook  # noqa: F401
        except ImportError:
            # profiling hook unavailable; run without trace (the env var is
            # re-read inside run_bass_kernel_spmd, so pin it off too)
            trace = False
            os.environ["BASS_NEVER_TRACE"] = "1"
    res = run_bass_kernel_spmd(nc, in_maps, core_ids=list(range(B)), trace=trace)
    LAST_RESULT = res
    r = res.results
    dist_enc = np.stack([r[b]["dist_out"] for b in range(B)]).astype(np.float32)
    frontier = np.stack([r[b]["front_out"] for b in range(B)]).astype(np.float32)
    agent = np.stack([r[b]["agent_out"] for b in range(B)]).astype(np.float32)
    phist = np.stack([r[b]["phist_out"] for b in range(B)]).astype(np.float32)
    ghist = np.stack([r[b]["ghist_out"] for b in range(B)]).astype(np.float32)
    return (dist_enc, frontier, agent, phist, ghist)


# revision 31
# speedup vs baseline: 1.4809x; 1.0572x over previous
"""Trainium2 Bass kernel for nn_Encoder_44238163149250.

Sharding: data-parallel over batch B=8 -> one batch element per NeuronCore.
Each core computes all five encoder outputs for its batch element.

Per-core structure (batch element b), dist and posenc chunks interleaved in
one Tile region so every engine stays loaded across the whole kernel:

  dist branch  (65536 pts, 512-pt chunks):
      y = relu(w1*log1p(d) + b1) @ W2.T + b2
      - log1p once on ACT over the whole [128,512] input; round-trip through
        a DRAM scratch so chunks re-load as [1,512] rows at base partition 0
        (SWDGE casts f32->fp16 during that reload)
      - broadcast across partitions with a K=1 ones outer-product matmul
      - h1 = Relu(scale*psum + bias) on ACT (per-partition scale/bias), fp16
      - layer 2 uses h1 slices as the matmul *stationary* operand
        (lhsT = h1[:,128j], rhs = W2.T fp16) so PSUM comes out in
        [points, feat] layout -> no transpose anywhere
      - bias-add + PSUM->SBUF on DVE; two chunks batched per output DMA
  posenc branches (frontier+ghist = fparams, phist+agent = aparams):
      - freq-scaled args via one K=3 fp16 matmul vs [idx_r; idx_c; 1]
        (integer idx and 2^k/512 coefficients are exact in fp16)
      - range-reduce to [-pi,pi]: k = round(x/2pi) via ACT Identity
        (scale+magic bias) + DVE subtract; r = x - k*fl32(2pi) via DVE
        scalar_tensor_tensor; clamp on DVE; cos rows are sin(x + pi/2)
        with the shift riding the ones row of the idx matmul
      - 3-layer fp16 MLP, layer biases folded into ACT per-partition bias,
        last layer via the same stationary-operand trick

  Point order inside each 512-pt chunk is host-permuted (position j*128+p
  holds point 4p+j) so each partition's DMA store is one contiguous 2KB run
  instead of 4 scattered 512B runs.
"""

import os

import numpy as np

import concourse.bass as bass
import concourse.mybir as mybir
import concourse.tile as tile
from concourse.bass_utils import run_bass_kernel_spmd

F32 = mybir.dt.float32
F16 = mybir.dt.float16
AF = mybir.ActivationFunctionType
ALU = mybir.AluOpType

NUM_FREQS = 10
B = 8

# per-core sizes
ND = 65536          # dist points (D*NF = 8*8192)
NFR = 8192          # frontier pts
NG = 2048           # ghist pts
NPH = 2048          # phist pts
NA = 8              # agent pts
NFB = NFR + NG      # F-branch pts  (fparams + type_frontier)
NAB = 2560          # A-branch pts padded (phist 2048 + agent 8 -> 2560)

PC = 512            # chunk size (points) for both branches
NDC = ND // PC      # 128 dist chunks
NFC = NFB // PC     # 20
NAC = NAB // PC     # 5

MAGIC = 12582912.0                       # 1.5 * 2**23: round-to-nearest trick
TWO_PI = 2.0 * np.pi
FL2PI = float(np.float32(TWO_PI))        # r = x - k*fl32(2pi): |err| <= 5e-5 rad
INV_2PI = 1.0 / TWO_PI
PI_LO = float(np.nextafter(np.float32(np.pi), np.float32(0)))  # f32 just below pi

_PROG = None        # cached nc build
_LEGALIZED = False  # wait-legalization applied to _PROG (HW path only)
LAST_RESULT = None  # BassKernelResults of the last kernel() call (for test.py)

DIST_ON = os.environ.get("K_DIST", "1") == "1"
POSENC_ON = os.environ.get("K_POSENC", "1") == "1"


def _legalize_waits(nc):
    """This toolchain's walrus accepts at most ONE sync wait per instruction
    (codegen raises 'Too many sync wait commands' otherwise), while Tile
    freely emits several. Hoist all but the last wait of each instruction
    onto same-engine EventSemaphore instructions inserted right before it —
    engines execute their queue in order, so sequential waits are equivalent
    to one multi-wait."""
    n = 0
    for f in nc.m.functions:
        for b in f.blocks:
            insts = list(b.instructions)
            out = []
            changed = False
            for i in insts:
                si = i.sync_info
                if si is not None and len(si.on_wait) > 1:
                    waits = list(si.on_wait)
                    for j, w in enumerate(waits[:-1]):
                        ev = mybir.InstEventSemaphore(
                            name=f"lw_{i.name}_{j}",
                            engine=i.engine,
                            ins=[],
                            outs=[],
                            sync_info=mybir.SyncInfo(on_wait=[w], on_update=[]),
                        )
                        out.append(ev)
                        n += 1
                    i.sync_info = mybir.SyncInfo(
                        on_wait=[waits[-1]], on_update=list(si.on_update)
                    )
                    changed = True
                out.append(i)
            if changed:
                cur = b.instructions
                del cur[:]
                for i in out:
                    cur.append(i)
    return n


def _build_program():
    nc = bass.Bass()

    # ---- DRAM I/O ----
    din = nc.dram_tensor("dist", [128, 512], F32, kind="ExternalInput")
    idx_f = nc.dram_tensor("idxf", [3, NFB], F16, kind="ExternalInput")
    idx_a = nc.dram_tensor("idxa", [3, NAB], F16, kind="ExternalInput")
    w1d = nc.dram_tensor("w1d", [128, 1], F32, kind="ExternalInput")
    b1d = nc.dram_tensor("b1d", [128, 1], F32, kind="ExternalInput")
    w2td = nc.dram_tensor("w2td", [128, 128], F16, kind="ExternalInput")
    b2bd = nc.dram_tensor("b2bd", [128, PC], F32, kind="ExternalInput")
    brt = {}
    for s in ("f", "a"):
        brt[s] = dict(
            w1a=nc.dram_tensor(f"w1a{s}", [40, 64], F16, kind="ExternalInput"),
            mm=nc.dram_tensor(f"mm{s}", [3, 128], F16, kind="ExternalInput"),
            w2t=nc.dram_tensor(f"w2t{s}", [64, 128], F16, kind="ExternalInput"),
            b2c=nc.dram_tensor(f"b2c{s}", [128, 1], F32, kind="ExternalInput"),
            w3t=nc.dram_tensor(f"w3t{s}", [128, 128], F16, kind="ExternalInput"),
            b3b=nc.dram_tensor(f"b3b{s}", [128, PC], F32, kind="ExternalInput"),
        )

    dout = nc.dram_tensor("dist_out", [ND, 128], F16, kind="ExternalOutput")
    fout = nc.dram_tensor("front_out", [NFR, 128], F16, kind="ExternalOutput")
    gout = nc.dram_tensor("ghist_out", [NG, 128], F16, kind="ExternalOutput")
    pout = nc.dram_tensor("phist_out", [NPH, 128], F16, kind="ExternalOutput")
    aout = nc.dram_tensor("agent_out", [NA, 128], F16, kind="ExternalOutput")

    # Views matching the in-chunk permutation: position j*128+p <-> point
    # 512c + 4p + j. Dist pairs two chunks per DMA.
    dview = dout.rearrange("(t h p g) o -> t p h g o", h=2, p=128, g=4)
    fview = fout.rearrange("(c p g) o -> c p g o", p=128, g=4)
    gview = gout.rearrange("(c p g) o -> c p g o", p=128, g=4)
    pview = pout.rearrange("(c p g) o -> c p g o", p=128, g=4)
    aview = aout.rearrange("(p j) o -> p j o", j=4)

    with tile.TileContext(nc) as tc:
        with tc.tile_pool(name="const", bufs=1) as cp:
            # dist input + log1p first so the scratch/ct chain starts before
            # the (many) constant loads occupy the queues
            dist_sb = cp.tile([128, 512], F32, tag="dist_sb")
            nc.sync.dma_start(out=dist_sb[:], in_=din[:, :])
            logd = cp.tile([128, 512], F32, tag="logd")
            nc.scalar.activation(logd[:], dist_sb[:], AF.Ln, bias=1.0, scale=1.0)

            def cload(dram, shape, tag, dt=F32):
                t = cp.tile(shape, dt, tag=tag)
                nc.sync.dma_start(out=t[:], in_=dram[:, :])
                return t

            w1d_t = cload(w1d, [128, 1], "w1d")
            b1d_t = cload(b1d, [128, 1], "b1d")
            w2td_t = cload(w2td, [128, 128], "w2td", F16)
            b2bd_t = cload(b2bd, [128, PC], "b2bd")
            brc = {}
            for s in ("f", "a"):
                d = brt[s]
                brc[s] = dict(
                    w1a=cload(d["w1a"], [40, 64], f"w1a{s}", F16),
                    mm=cload(d["mm"], [3, 128], f"mm{s}", F16),
                    w2t=cload(d["w2t"], [64, 128], f"w2t{s}", F16),
                    b2c=cload(d["b2c"], [128, 1], f"b2c{s}"),
                    w3t=cload(d["w3t"], [128, 128], f"w3t{s}", F16),
                    b3b=cload(d["b3b"], [128, PC], f"b3b{s}"),
                )
            idxf_t = cload(idx_f, [3, NFB], "idxf", F16)
            idxa_t = cload(idx_a, [3, NAB], "idxa", F16)

            ones_l = cp.tile([1, 128], F16, tag="ones")
            nc.vector.memset(ones_l[:], 1.0)
            magic_c = cp.tile([40, 1], F32, tag="magic")
            nc.vector.memset(magic_c[:], MAGIC)

            with (
                tc.tile_pool(name="ddram", bufs=1, space="DRAM") as ddram,
                tc.tile_pool(name="dsb", bufs=4) as dsb,
                tc.tile_pool(name="esb", bufs=4) as esb,
                tc.tile_pool(name="dpb", bufs=2, space="PSUM") as dpb,
                tc.tile_pool(name="dpo", bufs=2, space="PSUM") as dpo,
                tc.tile_pool(name="eps", bufs=2, space="PSUM") as eps,
                tc.tile_pool(name="ep2", bufs=1, space="PSUM") as ep2,
                tc.tile_pool(name="epo", bufs=1, space="PSUM") as epo,
            ):
                scr = ddram.tile([128, 512], F32, tag="scr")
                nc.gpsimd.dma_start(out=scr[:], in_=logd[:])

                def dist_chunk(c, ot_half):
                    ct = dsb.tile([1, PC], F16, tag="ct")
                    nc.gpsimd.dma_start(out=ct[:], in_=scr[c : c + 1, :])
                    psb = dpb.tile([128, PC], F32, tag="psb")
                    nc.tensor.matmul(
                        psb[:], lhsT=ones_l[:], rhs=ct[0:1, :],
                        start=True, stop=True,
                    )
                    h1 = dsb.tile([128, PC], F16, tag="h1")
                    nc.scalar.activation(
                        h1[:], psb[:], AF.Relu, bias=b1d_t[:], scale=w1d_t[:]
                    )
                    pso = dpo.tile([128, PC], F32, tag="pso")
                    for j in range(PC // 128):
                        nc.tensor.matmul(
                            pso[:, j * 128 : (j + 1) * 128],
                            lhsT=h1[:, j * 128 : (j + 1) * 128],
                            rhs=w2td_t[:],
                            start=True,
                            stop=True,
                        )
                    nc.vector.tensor_add(ot_half, pso[:], b2bd_t[:])

                def posenc_chunk(s, idx_t, c):
                    w = brc[s]
                    lo = c * PC
                    # One K=3 matmul vs [idx_r; idx_c; 1] produces BOTH the
                    # freq-scaled sin/cos args (psum rows 64..103) AND the
                    # layer-1 coords+bias partial W1b@idx + b1 (rows 0..63,
                    # bias riding the ones row); L1a later accumulates the
                    # sin/cos contribution onto rows 0..63 in the same bank.
                    pss = eps.tile([128, PC], F32, tag="pss")
                    nc.tensor.matmul(
                        pss[:], lhsT=w["mm"][:], rhs=idx_t[:, lo : lo + PC],
                        start=True, stop=True,
                    )
                    # k = round(args/2pi): scale+magic on ACT, subtract on DVE
                    kt1 = esb.tile([40, PC], F32, tag="kt1")
                    nc.scalar.activation(
                        kt1[:], pss[64:104, :], AF.Identity,
                        bias=magic_c[:], scale=INV_2PI,
                    )
                    kt = esb.tile([40, PC], F32, tag="kt")
                    nc.vector.tensor_scalar(
                        out=kt[:], in0=kt1[:], scalar1=MAGIC, scalar2=None,
                        op0=ALU.subtract,
                    )
                    rt = esb.tile([40, PC], F32, tag="rt")
                    nc.vector.scalar_tensor_tensor(
                        out=rt[:], in0=kt[:], scalar=-FL2PI,
                        in1=pss[64:104, :], op0=ALU.mult, op1=ALU.add,
                    )
                    r3 = esb.tile([40, PC], F32, tag="r3")
                    nc.vector.tensor_scalar(
                        out=r3[:], in0=rt[:],
                        scalar1=PI_LO, scalar2=-PI_LO,
                        op0=ALU.min, op1=ALU.max,
                    )
                    xs = esb.tile([40, PC], F16, tag="xs")
                    nc.scalar.activation(xs[:], r3[:], AF.Sin)

                    nc.tensor.matmul(
                        pss[0:64, :], lhsT=w["w1a"][:], rhs=xs[:],
                        start=False, stop=True, skip_group_check=True,
                    )
                    h1e = esb.tile([64, PC], F16, tag="h1e")
                    nc.scalar.activation(h1e[:], pss[0:64, :], AF.Relu)

                    ps2 = ep2.tile([128, PC], F32, tag="ps2")
                    nc.tensor.matmul(
                        ps2[:], lhsT=w["w2t"][:], rhs=h1e[:], start=True, stop=True
                    )
                    h2e = esb.tile([128, PC], F16, tag="h2e")
                    nc.scalar.activation(h2e[:], ps2[:], AF.Relu, bias=w["b2c"][:])

                    po = epo.tile([128, PC], F32, tag="po")
                    for j in range(PC // 128):
                        nc.tensor.matmul(
                            po[:, j * 128 : (j + 1) * 128],
                            lhsT=h2e[:, j * 128 : (j + 1) * 128],
                            rhs=w["w3t"][:],
                            start=True,
                            stop=True,
                        )
                    oute = esb.tile([128, PC], F16, tag="oute")
                    nc.vector.tensor_add(oute[:], po[:], w["b3b"][:])

                    src = oute[:].rearrange("p (g o) -> p g o", g=4)
                    if s == "f":
                        if c < NFR // PC:
                            nc.sync.dma_start(out=fview[c, :, :, :], in_=src)
                        else:
                            nc.sync.dma_start(
                                out=gview[c - NFR // PC, :, :, :], in_=src
                            )
                    else:
                        if c < NPH // PC:
                            nc.sync.dma_start(out=pview[c, :, :, :], in_=src)
                        else:
                            nc.sync.dma_start(
                                out=aview[:, :, :],
                                in_=oute[0:2, :].rearrange("p (j o) -> p j o", j=4),
                            )

                pe_jobs = []
                if POSENC_ON:
                    pe_jobs = [("f", idxf_t, c) for c in range(NFC)] + [
                        ("a", idxa_t, c) for c in range(NAC)
                    ]
                n_pairs = NDC // 2 if DIST_ON else 0
                emitted = 0
                # posenc has a much shorter input lead-in than dist (which
                # waits on the log1p -> DRAM scratch round-trip); start with a
                # few posenc chunks so the PE isn't idle at kernel start
                for _ in range(min(5, len(pe_jobs))):
                    posenc_chunk(*pe_jobs[emitted])
                    emitted += 1
                for t in range(n_pairs):
                    ot = dsb.tile([128, 2 * PC], F16, tag="ot")
                    for h in range(2):
                        dist_chunk(2 * t + h, ot[:, h * PC : (h + 1) * PC])
                    nc.sync.dma_start(
                        out=dview[t, :, :, :, :],
                        in_=ot[:].rearrange("p (h g o) -> p h g o", h=2, g=4),
                    )
                    want = (t + 1) * len(pe_jobs) // n_pairs
                    while emitted < want:
                        posenc_chunk(*pe_jobs[emitted])
                        emitted += 1
                while emitted < len(pe_jobs):
                    posenc_chunk(*pe_jobs[emitted])
                    emitted += 1
    return nc


def _get_program():
    global _PROG
    if _PROG is None:
        _PROG = _build_program()
    return _PROG


def _perm512(a):
    """Permute rows within each 512 block: new position j*128+p <- old 4p+j."""
    n = a.shape[0]
    assert n % 512 == 0
    rest = a.shape[1:]
    return np.ascontiguousarray(
        a.reshape(n // 512, 128, 4, *rest).swapaxes(1, 2).reshape(a.shape)
    )


def _host_inputs(inputs):
    """Build the per-core in_maps from the full problem inputs."""
    g = {k: np.asarray(v) for k, v in inputs.items()}
    sz = np.array([float(g["sz_r"]), float(g["sz_c"])], dtype=np.float64)

    def f32(x):
        return np.ascontiguousarray(np.asarray(x, dtype=np.float32))

    def f16(x):
        return np.ascontiguousarray(np.asarray(x, dtype=np.float16))

    # args columns of the per-branch scaled matmul (cols 64..103): fp16 is
    # exact for the 2^k/sz coefficients and integer idx; only the pi/2 shift
    # rounds (~5e-4 rad).
    Margs = np.zeros((3, 40), dtype=np.float64)
    for j in range(40):
        blk = j if j < 20 else j - 20
        k, cc = blk // 2, blk % 2
        Margs[cc, j] = (2.0**k) / sz[cc]
        Margs[2, j] = 0.0 if j < 20 else np.pi / 2

    Wd1, bd1 = g["Wd1"], g["bd1"]
    Wd2, bd2 = g["Wd2"], g["bd2"]
    consts = {
        "w1d": f32(Wd1[:, 0:1]),
        "b1d": f32(bd1.reshape(128, 1)),
        "w2td": f16(np.asarray(Wd2, np.float64).T),
        "b2bd": f32(np.tile(bd2.reshape(1, -1), (128, PC // 128))),
    }

    # posenc weights; X row order -> original pe column order
    sin_src = [2 + 4 * (i // 2) + (i % 2) for i in range(20)]
    cos_src = [2 + 4 * (i // 2) + 2 + (i % 2) for i in range(20)]
    cols = sin_src + cos_src
    for s, W1, b1, W2, b2, W3, b3, tv in (
        ("f", g["Wf1"], g["bf1"], g["Wf2"], g["bf2"], g["Wf3"], g["bf3"], (1.0, 0.0)),
        ("a", g["Wa1"], g["ba1"], g["Wa2"], g["ba2"], g["Wa3"], g["ba3"], (0.0, 1.0)),
    ):
        W1 = np.asarray(W1, np.float64)
        b1e = np.asarray(b1, np.float64) + W1[:, 42] * tv[0] + W1[:, 43] * tv[1]
        consts[f"w1a{s}"] = f16(W1[:, cols].T)               # [40, 64]
        Mb = np.zeros((3, 128), dtype=np.float64)
        Mb[0:2, 0:64] = (W1[:, 0:2] / sz[None, :]).T           # coords partial
        Mb[2, 0:64] = b1e                                      # bias via ones row
        Mb[:, 64:104] = Margs
        consts[f"mm{s}"] = f16(Mb)
        consts[f"w2t{s}"] = f16(np.asarray(W2, np.float64).T)  # [64, 128]
        consts[f"b2c{s}"] = f32(np.asarray(b2).reshape(128, 1))
        consts[f"w3t{s}"] = f16(np.asarray(W3, np.float64).T)  # [128, 128]
        consts[f"b3b{s}"] = f32(np.tile(np.asarray(b3).reshape(1, -1), (128, PC // 128)))

    in_maps = []
    fi, gi = g["frontier_idx"], g["ghistory_idx"]
    pi_, ai = g["phistory_idx"], g["agent_pos"]
    dv = g["dist_vals"]
    for b in range(B):
        idxf = np.concatenate([np.asarray(fi[b]), np.asarray(gi[b])], axis=0)
        idxa_raw = np.concatenate([np.asarray(pi_[b]), np.asarray(ai[b])], axis=0)
        idxa = np.zeros((NAB, 2), dtype=np.float64)
        idxa[: idxa_raw.shape[0]] = np.asarray(idxa_raw, np.float64)

        idxf_p = _perm512(np.asarray(idxf, np.float64))
        idxa_p = _perm512(idxa)

        def with_ones(t, n):
            out = np.ones((3, n), dtype=np.float16)
            out[0:2, :] = t.T.astype(np.float16)
            return np.ascontiguousarray(out)

        m = dict(consts)
        m["idxf"] = with_ones(idxf_p, NFB)
        m["idxa"] = with_ones(idxa_p, NAB)
        dperm = _perm512(np.asarray(dv[b], np.float32).reshape(ND))
        m["dist"] = f32(dperm.reshape(128, 512))
        in_maps.append(m)
    return in_maps


def kernel(**inputs):
    global LAST_RESULT, _LEGALIZED
    nc = _get_program()
    if not _LEGALIZED:
        # CoreSim can't execute the injected EventSemaphores, so this runs
        # only on the hardware path.
        _legalize_waits(nc)
        _LEGALIZED = True
    in_maps = _host_inputs(inputs)
    trace = os.environ.get("BASS_TRACE", "") not in ("", "0")
    if trace:
        try:
            from antenv.axon_hooks import get_axon_ntff_profile_hook  # noqa: F401
        except ImportError:
            # profiling hook unavailable; run without trace (the env var is
            # re-read inside run_bass_kernel_spmd, so pin it off too)
            trace = False
            os.environ["BASS_NEVER_TRACE"] = "1"
    res = run_bass_kernel_spmd(nc, in_maps, core_ids=list(range(B)), trace=trace)
    LAST_RESULT = res
    r = res.results
    dist_enc = np.stack([r[b]["dist_out"] for b in range(B)]).astype(np.float32)
    frontier = np.stack([r[b]["front_out"] for b in range(B)]).astype(np.float32)
    agent = np.stack([r[b]["agent_out"] for b in range(B)]).astype(np.float32)
    phist = np.stack([r[b]["phist_out"] for b in range(B)]).astype(np.float32)
    ghist = np.stack([r[b]["ghist_out"] for b in range(B)]).astype(np.float32)
    return (dist_enc, frontier, agent, phist, ghist)


# revision 32
# speedup vs baseline: 1.5114x; 1.0206x over previous
"""Trainium2 Bass kernel for nn_Encoder_44238163149250.

Sharding: data-parallel over batch B=8 -> one batch element per NeuronCore.
Each core computes all five encoder outputs for its batch element.

Per-core structure (batch element b), dist and posenc chunks interleaved in
one Tile region so every engine stays loaded across the whole kernel:

  dist branch  (65536 pts, 512-pt chunks):
      y = relu(w1*log1p(d) + b1) @ W2.T + b2
      - log1p once on ACT over the whole [128,512] input; round-trip through
        a DRAM scratch so chunks re-load as [1,512] rows at base partition 0
        (SWDGE casts f32->fp16 during that reload)
      - broadcast across partitions with a K=1 ones outer-product matmul
      - h1 = Relu(scale*psum + bias) on ACT (per-partition scale/bias), fp16
      - layer 2 uses h1 slices as the matmul *stationary* operand
        (lhsT = h1[:,128j], rhs = W2.T fp16) so PSUM comes out in
        [points, feat] layout -> no transpose anywhere
      - bias-add + PSUM->SBUF on DVE; two chunks batched per output DMA
  posenc branches (frontier+ghist = fparams, phist+agent = aparams):
      - freq-scaled args via one K=3 fp16 matmul vs [idx_r; idx_c; 1]
        (integer idx and 2^k/512 coefficients are exact in fp16)
      - range-reduce to [-pi,pi]: k = round(x/2pi) via ACT Identity
        (scale+magic bias) + DVE subtract; r = x - k*fl32(2pi) via DVE
        scalar_tensor_tensor; clamp on DVE; cos rows are sin(x + pi/2)
        with the shift riding the ones row of the idx matmul
      - 3-layer fp16 MLP, layer biases folded into ACT per-partition bias,
        last layer via the same stationary-operand trick

  Point order inside each 512-pt chunk is host-permuted (position j*128+p
  holds point 4p+j) so each partition's DMA store is one contiguous 2KB run
  instead of 4 scattered 512B runs.
"""

import os

import numpy as np

import concourse.bass as bass
import concourse.mybir as mybir
import concourse.tile as tile
from concourse.bass_utils import run_bass_kernel_spmd

F32 = mybir.dt.float32
F16 = mybir.dt.float16
AF = mybir.ActivationFunctionType
ALU = mybir.AluOpType

NUM_FREQS = 10
B = 8

# per-core sizes
ND = 65536          # dist points (D*NF = 8*8192)
NFR = 8192          # frontier pts
NG = 2048           # ghist pts
NPH = 2048          # phist pts
NA = 8              # agent pts
NFB = NFR + NG      # F-branch pts  (fparams + type_frontier)
NAB = 2560          # A-branch pts padded (phist 2048 + agent 8 -> 2560)

PC = 512            # chunk size (points) for both branches
NDC = ND // PC      # 128 dist chunks
NFC = NFB // PC     # 20
NAC = NAB // PC     # 5

MAGIC = 12582912.0                       # 1.5 * 2**23: round-to-nearest trick
TWO_PI = 2.0 * np.pi
FL2PI = float(np.float32(TWO_PI))        # r = x - k*fl32(2pi): |err| <= 5e-5 rad
INV_2PI = 1.0 / TWO_PI
PI_LO = float(np.nextafter(np.float32(np.pi), np.float32(0)))  # f32 just below pi

_PROG = None        # cached nc build
_LEGALIZED = False  # wait-legalization applied to _PROG (HW path only)
LAST_RESULT = None  # BassKernelResults of the last kernel() call (for test.py)

DIST_ON = os.environ.get("K_DIST", "1") == "1"
POSENC_ON = os.environ.get("K_POSENC", "1") == "1"


def _legalize_waits(nc):
    """This toolchain's walrus accepts at most ONE sync wait per instruction
    (codegen raises 'Too many sync wait commands' otherwise), while Tile
    freely emits several. Hoist all but the last wait of each instruction
    onto same-engine EventSemaphore instructions inserted right before it —
    engines execute their queue in order, so sequential waits are equivalent
    to one multi-wait."""
    n = 0
    for f in nc.m.functions:
        for b in f.blocks:
            insts = list(b.instructions)
            out = []
            changed = False
            for i in insts:
                si = i.sync_info
                if si is not None and len(si.on_wait) > 1:
                    waits = list(si.on_wait)
                    for j, w in enumerate(waits[:-1]):
                        ev = mybir.InstEventSemaphore(
                            name=f"lw_{i.name}_{j}",
                            engine=i.engine,
                            ins=[],
                            outs=[],
                            sync_info=mybir.SyncInfo(on_wait=[w], on_update=[]),
                        )
                        out.append(ev)
                        n += 1
                    i.sync_info = mybir.SyncInfo(
                        on_wait=[waits[-1]], on_update=list(si.on_update)
                    )
                    changed = True
                out.append(i)
            if changed:
                cur = b.instructions
                del cur[:]
                for i in out:
                    cur.append(i)
    return n


def _build_program():
    nc = bass.Bass()

    # ---- DRAM I/O ----
    din = nc.dram_tensor("dist", [128, 512], F32, kind="ExternalInput")
    idx_f = nc.dram_tensor("idxf", [3, NFB], F16, kind="ExternalInput")
    idx_a = nc.dram_tensor("idxa", [3, NAB], F16, kind="ExternalInput")
    w1d = nc.dram_tensor("w1d", [128, 1], F32, kind="ExternalInput")
    b1d = nc.dram_tensor("b1d", [128, 1], F32, kind="ExternalInput")
    w2td = nc.dram_tensor("w2td", [128, 128], F16, kind="ExternalInput")
    b2bd = nc.dram_tensor("b2bd", [128, PC], F32, kind="ExternalInput")
    brt = {}
    for s in ("f", "a"):
        brt[s] = dict(
            w1a=nc.dram_tensor(f"w1a{s}", [40, 64], F16, kind="ExternalInput"),
            mm=nc.dram_tensor(f"mm{s}", [3, 128], F16, kind="ExternalInput"),
            w2t=nc.dram_tensor(f"w2t{s}", [64, 128], F16, kind="ExternalInput"),
            b2c=nc.dram_tensor(f"b2c{s}", [128, 1], F32, kind="ExternalInput"),
            w3t=nc.dram_tensor(f"w3t{s}", [128, 128], F16, kind="ExternalInput"),
            b3b=nc.dram_tensor(f"b3b{s}", [128, PC], F32, kind="ExternalInput"),
        )

    dout = nc.dram_tensor("dist_out", [ND, 128], F16, kind="ExternalOutput")
    fout = nc.dram_tensor("front_out", [NFR, 128], F16, kind="ExternalOutput")
    gout = nc.dram_tensor("ghist_out", [NG, 128], F16, kind="ExternalOutput")
    pout = nc.dram_tensor("phist_out", [NPH, 128], F16, kind="ExternalOutput")
    aout = nc.dram_tensor("agent_out", [NA, 128], F16, kind="ExternalOutput")

    # Views matching the in-chunk permutation: position j*128+p <-> point
    # 512c + 4p + j. Dist pairs two chunks per DMA.
    dview = dout.rearrange("(t h p g) o -> t p h g o", h=2, p=128, g=4)
    fview = fout.rearrange("(c p g) o -> c p g o", p=128, g=4)
    gview = gout.rearrange("(c p g) o -> c p g o", p=128, g=4)
    pview = pout.rearrange("(c p g) o -> c p g o", p=128, g=4)
    aview = aout.rearrange("(p j) o -> p j o", j=4)

    with tile.TileContext(nc) as tc:
        with tc.tile_pool(name="const", bufs=1) as cp:
            # dist input + log1p first so the scratch/ct chain starts before
            # the (many) constant loads occupy the queues
            dist_sb = cp.tile([128, 512], F32, tag="dist_sb")
            nc.sync.dma_start(out=dist_sb[:], in_=din[:, :])
            logd = cp.tile([128, 512], F32, tag="logd")
            nc.scalar.activation(logd[:], dist_sb[:], AF.Ln, bias=1.0, scale=1.0)

            def cload(dram, shape, tag, dt=F32):
                t = cp.tile(shape, dt, tag=tag)
                nc.sync.dma_start(out=t[:], in_=dram[:, :])
                return t

            w1d_t = cload(w1d, [128, 1], "w1d")
            b1d_t = cload(b1d, [128, 1], "b1d")
            w2td_t = cload(w2td, [128, 128], "w2td", F16)
            b2bd_t = cload(b2bd, [128, PC], "b2bd")
            brc = {}
            for s in ("f", "a"):
                d = brt[s]
                brc[s] = dict(
                    w1a=cload(d["w1a"], [40, 64], f"w1a{s}", F16),
                    mm=cload(d["mm"], [3, 128], f"mm{s}", F16),
                    w2t=cload(d["w2t"], [64, 128], f"w2t{s}", F16),
                    b2c=cload(d["b2c"], [128, 1], f"b2c{s}"),
                    w3t=cload(d["w3t"], [128, 128], f"w3t{s}", F16),
                    b3b=cload(d["b3b"], [128, PC], f"b3b{s}"),
                )
            idxf_t = cload(idx_f, [3, NFB], "idxf", F16)
            idxa_t = cload(idx_a, [3, NAB], "idxa", F16)

            ones_l = cp.tile([1, 128], F16, tag="ones")
            nc.vector.memset(ones_l[:], 1.0)
            magic_c = cp.tile([40, 1], F32, tag="magic")
            nc.vector.memset(magic_c[:], MAGIC)

            with (
                tc.tile_pool(name="ddram", bufs=1, space="DRAM") as ddram,
                tc.tile_pool(name="dsb", bufs=6) as dsb,
                tc.tile_pool(name="esb", bufs=6) as esb,
                tc.tile_pool(name="dpb", bufs=2, space="PSUM") as dpb,
                tc.tile_pool(name="dpo", bufs=2, space="PSUM") as dpo,
                tc.tile_pool(name="eps", bufs=2, space="PSUM") as eps,
                tc.tile_pool(name="ep2", bufs=1, space="PSUM") as ep2,
                tc.tile_pool(name="epo", bufs=1, space="PSUM") as epo,
            ):
                scr = ddram.tile([128, 512], F32, tag="scr")
                nc.gpsimd.dma_start(out=scr[:], in_=logd[:])

                def dist_chunk(c, ct_row, ot_half):
                    psb = dpb.tile([128, PC], F32, tag="psb")
                    nc.tensor.matmul(
                        psb[:], lhsT=ones_l[:], rhs=ct_row,
                        start=True, stop=True,
                    )
                    h1 = dsb.tile([128, PC], F16, tag="h1")
                    nc.scalar.activation(
                        h1[:], psb[:], AF.Relu, bias=b1d_t[:], scale=w1d_t[:]
                    )
                    pso = dpo.tile([128, PC], F32, tag="pso")
                    for j in range(PC // 128):
                        nc.tensor.matmul(
                            pso[:, j * 128 : (j + 1) * 128],
                            lhsT=h1[:, j * 128 : (j + 1) * 128],
                            rhs=w2td_t[:],
                            start=True,
                            stop=True,
                        )
                    nc.vector.tensor_add(ot_half, pso[:], b2bd_t[:])

                def posenc_chunk(s, idx_t, c):
                    w = brc[s]
                    lo = c * PC
                    # One K=3 matmul vs [idx_r; idx_c; 1] produces BOTH the
                    # freq-scaled sin/cos args (psum rows 64..103) AND the
                    # layer-1 coords+bias partial W1b@idx + b1 (rows 0..63,
                    # bias riding the ones row); L1a later accumulates the
                    # sin/cos contribution onto rows 0..63 in the same bank.
                    pss = eps.tile([128, PC], F32, tag="pss")
                    nc.tensor.matmul(
                        pss[:], lhsT=w["mm"][:], rhs=idx_t[:, lo : lo + PC],
                        start=True, stop=True,
                    )
                    # k = round(args/2pi): scale+magic on ACT, subtract on DVE
                    kt1 = esb.tile([40, PC], F32, tag="kt1")
                    nc.scalar.activation(
                        kt1[:], pss[64:104, :], AF.Identity,
                        bias=magic_c[:], scale=INV_2PI,
                    )
                    kt = esb.tile([40, PC], F32, tag="kt")
                    nc.vector.tensor_scalar(
                        out=kt[:], in0=kt1[:], scalar1=MAGIC, scalar2=None,
                        op0=ALU.subtract,
                    )
                    rt = esb.tile([40, PC], F32, tag="rt")
                    nc.vector.scalar_tensor_tensor(
                        out=rt[:], in0=kt[:], scalar=-FL2PI,
                        in1=pss[64:104, :], op0=ALU.mult, op1=ALU.add,
                    )
                    r3 = esb.tile([40, PC], F32, tag="r3")
                    nc.vector.tensor_scalar(
                        out=r3[:], in0=rt[:],
                        scalar1=PI_LO, scalar2=-PI_LO,
                        op0=ALU.min, op1=ALU.max,
                    )
                    xs = esb.tile([40, PC], F16, tag="xs")
                    nc.scalar.activation(xs[:], r3[:], AF.Sin)

                    nc.tensor.matmul(
                        pss[0:64, :], lhsT=w["w1a"][:], rhs=xs[:],
                        start=False, stop=True, skip_group_check=True,
                    )
                    h1e = esb.tile([64, PC], F16, tag="h1e")
                    nc.scalar.activation(h1e[:], pss[0:64, :], AF.Relu)

                    ps2 = ep2.tile([128, PC], F32, tag="ps2")
                    nc.tensor.matmul(
                        ps2[:], lhsT=w["w2t"][:], rhs=h1e[:], start=True, stop=True
                    )
                    h2e = esb.tile([128, PC], F16, tag="h2e")
                    nc.scalar.activation(h2e[:], ps2[:], AF.Relu, bias=w["b2c"][:])

                    po = epo.tile([128, PC], F32, tag="po")
                    for j in range(PC // 128):
                        nc.tensor.matmul(
                            po[:, j * 128 : (j + 1) * 128],
                            lhsT=h2e[:, j * 128 : (j + 1) * 128],
                            rhs=w["w3t"][:],
                            start=True,
                            stop=True,
                        )
                    oute = esb.tile([128, PC], F16, tag="oute")
                    nc.vector.tensor_add(oute[:], po[:], w["b3b"][:])

                    src = oute[:].rearrange("p (g o) -> p g o", g=4)
                    if s == "f":
                        if c < NFR // PC:
                            nc.sync.dma_start(out=fview[c, :, :, :], in_=src)
                        else:
                            nc.sync.dma_start(
                                out=gview[c - NFR // PC, :, :, :], in_=src
                            )
                    else:
                        if c < NPH // PC:
                            nc.sync.dma_start(out=pview[c, :, :, :], in_=src)
                        else:
                            nc.sync.dma_start(
                                out=aview[:, :, :],
                                in_=oute[0:2, :].rearrange("p (j o) -> p j o", j=4),
                            )

                pe_jobs = []
                if POSENC_ON:
                    pe_jobs = [("f", idxf_t, c) for c in range(NFC)] + [
                        ("a", idxa_t, c) for c in range(NAC)
                    ]
                n_pairs = NDC // 2 if DIST_ON else 0
                emitted = 0
                # posenc has a much shorter input lead-in than dist (which
                # waits on the log1p -> DRAM scratch round-trip); start with a
                # few posenc chunks so the PE isn't idle at kernel start
                for _ in range(min(5, len(pe_jobs))):
                    posenc_chunk(*pe_jobs[emitted])
                    emitted += 1
                for t in range(n_pairs):
                    ot = dsb.tile([128, 2 * PC], F16, tag="ot")
                    ct = dsb.tile([1, 2 * PC], F16, tag="ct")
                    nc.gpsimd.dma_start(out=ct[:], in_=scr[2 * t : 2 * t + 2, :])
                    for h in range(2):
                        dist_chunk(
                            2 * t + h,
                            ct[0:1, h * PC : (h + 1) * PC],
                            ot[:, h * PC : (h + 1) * PC],
                        )
                    nc.sync.dma_start(
                        out=dview[t, :, :, :, :],
                        in_=ot[:].rearrange("p (h g o) -> p h g o", h=2, g=4),
                    )
                    want = (t + 1) * len(pe_jobs) // n_pairs
                    while emitted < want:
                        posenc_chunk(*pe_jobs[emitted])
                        emitted += 1
                while emitted < len(pe_jobs):
                    posenc_chunk(*pe_jobs[emitted])
                    emitted += 1
    return nc


def _get_program():
    global _PROG
    if _PROG is None:
        _PROG = _build_program()
    return _PROG


def _perm512(a):
    """Permute rows within each 512 block: new position j*128+p <- old 4p+j."""
    n = a.shape[0]
    assert n % 512 == 0
    rest = a.shape[1:]
    return np.ascontiguousarray(
        a.reshape(n // 512, 128, 4, *rest).swapaxes(1, 2).reshape(a.shape)
    )


def _host_inputs(inputs):
    """Build the per-core in_maps from the full problem inputs."""
    g = {k: np.asarray(v) for k, v in inputs.items()}
    sz = np.array([float(g["sz_r"]), float(g["sz_c"])], dtype=np.float64)

    def f32(x):
        return np.ascontiguousarray(np.asarray(x, dtype=np.float32))

    def f16(x):
        return np.ascontiguousarray(np.asarray(x, dtype=np.float16))

    # args columns of the per-branch scaled matmul (cols 64..103): fp16 is
    # exact for the 2^k/sz coefficients and integer idx; only the pi/2 shift
    # rounds (~5e-4 rad).
    Margs = np.zeros((3, 40), dtype=np.float64)
    for j in range(40):
        blk = j if j < 20 else j - 20
        k, cc = blk // 2, blk % 2
        Margs[cc, j] = (2.0**k) / sz[cc]
        Margs[2, j] = 0.0 if j < 20 else np.pi / 2

    Wd1, bd1 = g["Wd1"], g["bd1"]
    Wd2, bd2 = g["Wd2"], g["bd2"]
    consts = {
        "w1d": f32(Wd1[:, 0:1]),
        "b1d": f32(bd1.reshape(128, 1)),
        "w2td": f16(np.asarray(Wd2, np.float64).T),
        "b2bd": f32(np.tile(bd2.reshape(1, -1), (128, PC // 128))),
    }

    # posenc weights; X row order -> original pe column order
    sin_src = [2 + 4 * (i // 2) + (i % 2) for i in range(20)]
    cos_src = [2 + 4 * (i // 2) + 2 + (i % 2) for i in range(20)]
    cols = sin_src + cos_src
    for s, W1, b1, W2, b2, W3, b3, tv in (
        ("f", g["Wf1"], g["bf1"], g["Wf2"], g["bf2"], g["Wf3"], g["bf3"], (1.0, 0.0)),
        ("a", g["Wa1"], g["ba1"], g["Wa2"], g["ba2"], g["Wa3"], g["ba3"], (0.0, 1.0)),
    ):
        W1 = np.asarray(W1, np.float64)
        b1e = np.asarray(b1, np.float64) + W1[:, 42] * tv[0] + W1[:, 43] * tv[1]
        consts[f"w1a{s}"] = f16(W1[:, cols].T)               # [40, 64]
        Mb = np.zeros((3, 128), dtype=np.float64)
        Mb[0:2, 0:64] = (W1[:, 0:2] / sz[None, :]).T           # coords partial
        Mb[2, 0:64] = b1e                                      # bias via ones row
        Mb[:, 64:104] = Margs
        consts[f"mm{s}"] = f16(Mb)
        consts[f"w2t{s}"] = f16(np.asarray(W2, np.float64).T)  # [64, 128]
        consts[f"b2c{s}"] = f32(np.asarray(b2).reshape(128, 1))
        consts[f"w3t{s}"] = f16(np.asarray(W3, np.float64).T)  # [128, 128]
        consts[f"b3b{s}"] = f32(np.tile(np.asarray(b3).reshape(1, -1), (128, PC // 128)))

    in_maps = []
    fi, gi = g["frontier_idx"], g["ghistory_idx"]
    pi_, ai = g["phistory_idx"], g["agent_pos"]
    dv = g["dist_vals"]
    for b in range(B):
        idxf = np.concatenate([np.asarray(fi[b]), np.asarray(gi[b])], axis=0)
        idxa_raw = np.concatenate([np.asarray(pi_[b]), np.asarray(ai[b])], axis=0)
        idxa = np.zeros((NAB, 2), dtype=np.float64)
        idxa[: idxa_raw.shape[0]] = np.asarray(idxa_raw, np.float64)

        idxf_p = _perm512(np.asarray(idxf, np.float64))
        idxa_p = _perm512(idxa)

        def with_ones(t, n):
            out = np.ones((3, n), dtype=np.float16)
            out[0:2, :] = t.T.astype(np.float16)
            return np.ascontiguousarray(out)

        m = dict(consts)
        m["idxf"] = with_ones(idxf_p, NFB)
        m["idxa"] = with_ones(idxa_p, NAB)
        dperm = _perm512(np.asarray(dv[b], np.float32).reshape(ND))
        m["dist"] = f32(dperm.reshape(128, 512))
        in_maps.append(m)
    return in_maps


def kernel(**inputs):
    global LAST_RESULT, _LEGALIZED
    nc = _get_program()
    if not _LEGALIZED:
        # CoreSim can't execute the injected EventSemaphores, so this runs
        # only on the hardware path.
        _legalize_waits(nc)
        _LEGALIZED = True
    in_maps = _host_inputs(inputs)
    trace = os.environ.get("BASS_TRACE", "") not in ("", "0")
    if trace:
        try:
            from antenv.axon_hooks import get_axon_ntff_profile_hook  # noqa: F401
        except ImportError:
            # profiling hook unavailable; run without trace (the env var is
            # re-read inside run_bass_kernel_spmd, so pin it off too)
            trace = False
            os.environ["BASS_NEVER_TRACE"] = "1"
    res = run_bass_kernel_spmd(nc, in_maps, core_ids=list(range(B)), trace=trace)
    LAST_RESULT = res
    r = res.results
    dist_enc = np.stack([r[b]["dist_out"] for b in range(B)]).astype(np.float32)
    frontier = np.stack([r[b]["front_out"] for b in range(B)]).astype(np.float32)
    agent = np.stack([r[b]["agent_out"] for b in range(B)]).astype(np.float32)
    phist = np.stack([r[b]["phist_out"] for b in range(B)]).astype(np.float32)
    ghist = np.stack([r[b]["ghist_out"] for b in range(B)]).astype(np.float32)
    return (dist_enc, frontier, agent, phist, ghist)


# revision 34
# speedup vs baseline: 1.5203x; 1.0059x over previous
"""Trainium2 Bass kernel for nn_Encoder_44238163149250.

Sharding: data-parallel over batch B=8 -> one batch element per NeuronCore.
Each core computes all five encoder outputs for its batch element.

Per-core structure (batch element b), dist and posenc chunks interleaved in
one Tile region so every engine stays loaded across the whole kernel:

  dist branch  (65536 pts, 512-pt chunks):
      y = relu(w1*log1p(d) + b1) @ W2.T + b2
      - log1p once on ACT over the whole [128,512] input; round-trip through
        a DRAM scratch so chunks re-load as [1,512] rows at base partition 0
        (SWDGE casts f32->fp16 during that reload)
      - broadcast across partitions with a K=1 ones outer-product matmul
      - h1 = Relu(scale*psum + bias) on ACT (per-partition scale/bias), fp16
      - layer 2 uses h1 slices as the matmul *stationary* operand
        (lhsT = h1[:,128j], rhs = W2.T fp16) so PSUM comes out in
        [points, feat] layout -> no transpose anywhere
      - bias-add + PSUM->SBUF on DVE; two chunks batched per output DMA
  posenc branches (frontier+ghist = fparams, phist+agent = aparams):
      - freq-scaled args via one K=3 fp16 matmul vs [idx_r; idx_c; 1]
        (integer idx and 2^k/512 coefficients are exact in fp16)
      - range-reduce to [-pi,pi]: k = round(x/2pi) via ACT Identity
        (scale+magic bias) + DVE subtract; r = x - k*fl32(2pi) via DVE
        scalar_tensor_tensor; clamp on DVE; cos rows are sin(x + pi/2)
        with the shift riding the ones row of the idx matmul
      - 3-layer fp16 MLP, layer biases folded into ACT per-partition bias,
        last layer via the same stationary-operand trick

  Point order inside each 512-pt chunk is host-permuted (position j*128+p
  holds point 4p+j) so each partition's DMA store is one contiguous 2KB run
  instead of 4 scattered 512B runs.
"""

import os

import numpy as np

import concourse.bass as bass
import concourse.mybir as mybir
import concourse.tile as tile
from concourse.bass_utils import run_bass_kernel_spmd

F32 = mybir.dt.float32
F16 = mybir.dt.float16
AF = mybir.ActivationFunctionType
ALU = mybir.AluOpType

NUM_FREQS = 10
B = 8

# per-core sizes
ND = 65536          # dist points (D*NF = 8*8192)
NFR = 8192          # frontier pts
NG = 2048           # ghist pts
NPH = 2048          # phist pts
NA = 8              # agent pts
NFB = NFR + NG      # F-branch pts  (fparams + type_frontier)
NAB = 2560          # A-branch pts padded (phist 2048 + agent 8 -> 2560)

PC = 512            # chunk size (points) for both branches
NDC = ND // PC      # 128 dist chunks
NFC = NFB // PC     # 20
NAC = NAB // PC     # 5

MAGIC = 12582912.0                       # 1.5 * 2**23: round-to-nearest trick
TWO_PI = 2.0 * np.pi
FL2PI = float(np.float32(TWO_PI))        # r = x - k*fl32(2pi): |err| <= 5e-5 rad
INV_2PI = 1.0 / TWO_PI
PI_LO = float(np.nextafter(np.float32(np.pi), np.float32(0)))  # f32 just below pi

_PROG = None        # cached nc build
_LEGALIZED = False  # wait-legalization applied to _PROG (HW path only)
LAST_RESULT = None  # BassKernelResults of the last kernel() call (for test.py)

DIST_ON = os.environ.get("K_DIST", "1") == "1"
POSENC_ON = os.environ.get("K_POSENC", "1") == "1"


def _legalize_waits(nc):
    """This toolchain's walrus accepts at most ONE sync wait per instruction
    (codegen raises 'Too many sync wait commands' otherwise), while Tile
    freely emits several. Hoist all but the last wait of each instruction
    onto same-engine EventSemaphore instructions inserted right before it —
    engines execute their queue in order, so sequential waits are equivalent
    to one multi-wait."""
    n = 0
    for f in nc.m.functions:
        for b in f.blocks:
            insts = list(b.instructions)
            out = []
            changed = False
            for i in insts:
                si = i.sync_info
                if si is not None and len(si.on_wait) > 1:
                    waits = list(si.on_wait)
                    for j, w in enumerate(waits[:-1]):
                        ev = mybir.InstEventSemaphore(
                            name=f"lw_{i.name}_{j}",
                            engine=i.engine,
                            ins=[],
                            outs=[],
                            sync_info=mybir.SyncInfo(on_wait=[w], on_update=[]),
                        )
                        out.append(ev)
                        n += 1
                    i.sync_info = mybir.SyncInfo(
                        on_wait=[waits[-1]], on_update=list(si.on_update)
                    )
                    changed = True
                out.append(i)
            if changed:
                cur = b.instructions
                del cur[:]
                for i in out:
                    cur.append(i)
    return n


def _build_program():
    nc = bass.Bass()

    # ---- DRAM I/O ----
    din = nc.dram_tensor("dist", [128, 512], F32, kind="ExternalInput")
    idx_f = nc.dram_tensor("idxf", [3, NFB], F16, kind="ExternalInput")
    idx_a = nc.dram_tensor("idxa", [3, NAB], F16, kind="ExternalInput")
    w1d = nc.dram_tensor("w1d", [128, 1], F32, kind="ExternalInput")
    b1d = nc.dram_tensor("b1d", [128, 1], F32, kind="ExternalInput")
    w2td = nc.dram_tensor("w2td", [128, 128], F16, kind="ExternalInput")
    b2bd = nc.dram_tensor("b2bd", [128, PC], F32, kind="ExternalInput")
    brt = {}
    for s in ("f", "a"):
        brt[s] = dict(
            w1a=nc.dram_tensor(f"w1a{s}", [40, 64], F16, kind="ExternalInput"),
            mm=nc.dram_tensor(f"mm{s}", [3, 128], F16, kind="ExternalInput"),
            w2t=nc.dram_tensor(f"w2t{s}", [64, 128], F16, kind="ExternalInput"),
            b2c=nc.dram_tensor(f"b2c{s}", [128, 1], F32, kind="ExternalInput"),
            w3t=nc.dram_tensor(f"w3t{s}", [128, 128], F16, kind="ExternalInput"),
            b3b=nc.dram_tensor(f"b3b{s}", [128, PC], F32, kind="ExternalInput"),
        )

    dout = nc.dram_tensor("dist_out", [ND, 128], F16, kind="ExternalOutput")
    fout = nc.dram_tensor("front_out", [NFR, 128], F16, kind="ExternalOutput")
    gout = nc.dram_tensor("ghist_out", [NG, 128], F16, kind="ExternalOutput")
    pout = nc.dram_tensor("phist_out", [NPH, 128], F16, kind="ExternalOutput")
    aout = nc.dram_tensor("agent_out", [NA, 128], F16, kind="ExternalOutput")

    # Views matching the in-chunk permutation: position j*128+p <-> point
    # 512c + 4p + j. Dist pairs two chunks per DMA.
    dview = dout.rearrange("(t h p g) o -> t p h g o", h=2, p=128, g=4)
    fview = fout.rearrange("(c p g) o -> c p g o", p=128, g=4)
    gview = gout.rearrange("(c p g) o -> c p g o", p=128, g=4)
    pview = pout.rearrange("(c p g) o -> c p g o", p=128, g=4)
    aview = aout.rearrange("(p j) o -> p j o", j=4)

    with tile.TileContext(nc) as tc:
        with tc.tile_pool(name="const", bufs=1) as cp:
            # dist input + log1p first so the scratch/ct chain starts before
            # the (many) constant loads occupy the queues
            dist_sb = cp.tile([128, 512], F32, tag="dist_sb")
            nc.sync.dma_start(out=dist_sb[:], in_=din[:, :])
            logd = cp.tile([128, 512], F32, tag="logd")
            nc.scalar.activation(logd[:], dist_sb[:], AF.Ln, bias=1.0, scale=1.0)

            def cload(dram, shape, tag, dt=F32):
                t = cp.tile(shape, dt, tag=tag)
                nc.sync.dma_start(out=t[:], in_=dram[:, :])
                return t

            idxf_t = cload(idx_f, [3, NFB], "idxf", F16)
            idxa_t = cload(idx_a, [3, NAB], "idxa", F16)
            w1d_t = cload(w1d, [128, 1], "w1d")
            b1d_t = cload(b1d, [128, 1], "b1d")
            w2td_t = cload(w2td, [128, 128], "w2td", F16)
            b2bd_t = cload(b2bd, [128, PC], "b2bd")
            brc = {}
            for s in ("f", "a"):
                d = brt[s]
                brc[s] = dict(
                    w1a=cload(d["w1a"], [40, 64], f"w1a{s}", F16),
                    mm=cload(d["mm"], [3, 128], f"mm{s}", F16),
                    w2t=cload(d["w2t"], [64, 128], f"w2t{s}", F16),
                    b2c=cload(d["b2c"], [128, 1], f"b2c{s}"),
                    w3t=cload(d["w3t"], [128, 128], f"w3t{s}", F16),
                    b3b=cload(d["b3b"], [128, PC], f"b3b{s}"),
                )

            ones_l = cp.tile([1, 128], F16, tag="ones")
            nc.vector.memset(ones_l[:], 1.0)
            magic_c = cp.tile([40, 1], F32, tag="magic")
            nc.vector.memset(magic_c[:], MAGIC)

            with (
                tc.tile_pool(name="ddram", bufs=1, space="DRAM") as ddram,
                tc.tile_pool(name="dsb", bufs=6) as dsb,
                tc.tile_pool(name="esb", bufs=6) as esb,
                tc.tile_pool(name="dpb", bufs=2, space="PSUM") as dpb,
                tc.tile_pool(name="dpo", bufs=2, space="PSUM") as dpo,
                tc.tile_pool(name="eps", bufs=2, space="PSUM") as eps,
                tc.tile_pool(name="ep2", bufs=1, space="PSUM") as ep2,
                tc.tile_pool(name="epo", bufs=1, space="PSUM") as epo,
            ):
                scr = ddram.tile([128, 512], F32, tag="scr")
                nc.gpsimd.dma_start(out=scr[:], in_=logd[:])

                def dist_chunk(c, ct_row, ot_half):
                    psb = dpb.tile([128, PC], F32, tag="psb")
                    nc.tensor.matmul(
                        psb[:], lhsT=ones_l[:], rhs=ct_row,
                        start=True, stop=True,
                    )
                    h1 = dsb.tile([128, PC], F16, tag="h1")
                    nc.scalar.activation(
                        h1[:], psb[:], AF.Relu, bias=b1d_t[:], scale=w1d_t[:]
                    )
                    pso = dpo.tile([128, PC], F32, tag="pso")
                    for j in range(PC // 128):
                        nc.tensor.matmul(
                            pso[:, j * 128 : (j + 1) * 128],
                            lhsT=h1[:, j * 128 : (j + 1) * 128],
                            rhs=w2td_t[:],
                            start=True,
                            stop=True,
                        )
                    nc.vector.tensor_add(ot_half, pso[:], b2bd_t[:])

                def posenc_chunk(s, idx_t, c):
                    w = brc[s]
                    lo = c * PC
                    # One K=3 matmul vs [idx_r; idx_c; 1] produces BOTH the
                    # freq-scaled sin/cos args (psum rows 64..103) AND the
                    # layer-1 coords+bias partial W1b@idx + b1 (rows 0..63,
                    # bias riding the ones row); L1a later accumulates the
                    # sin/cos contribution onto rows 0..63 in the same bank.
                    pss = eps.tile([128, PC], F32, tag="pss")
                    nc.tensor.matmul(
                        pss[:], lhsT=w["mm"][:], rhs=idx_t[:, lo : lo + PC],
                        start=True, stop=True,
                    )
                    # k = round(args/2pi): scale+magic on ACT, subtract on DVE
                    kt1 = esb.tile([40, PC], F32, tag="kt1")
                    nc.scalar.activation(
                        kt1[:], pss[64:104, :], AF.Identity,
                        bias=magic_c[:], scale=INV_2PI,
                    )
                    kt = esb.tile([40, PC], F32, tag="kt")
                    nc.vector.tensor_scalar(
                        out=kt[:], in0=kt1[:], scalar1=MAGIC, scalar2=None,
                        op0=ALU.subtract,
                    )
                    rt = esb.tile([40, PC], F32, tag="rt")
                    nc.vector.scalar_tensor_tensor(
                        out=rt[:], in0=kt[:], scalar=-FL2PI,
                        in1=pss[64:104, :], op0=ALU.mult, op1=ALU.add,
                    )
                    r3 = esb.tile([40, PC], F32, tag="r3")
                    nc.vector.tensor_scalar(
                        out=r3[:], in0=rt[:],
                        scalar1=PI_LO, scalar2=-PI_LO,
                        op0=ALU.min, op1=ALU.max,
                    )
                    xs = esb.tile([40, PC], F16, tag="xs")
                    nc.scalar.activation(xs[:], r3[:], AF.Sin)

                    nc.tensor.matmul(
                        pss[0:64, :], lhsT=w["w1a"][:], rhs=xs[:],
                        start=False, stop=True, skip_group_check=True,
                    )
                    h1e = esb.tile([64, PC], F16, tag="h1e")
                    nc.scalar.activation(h1e[:], pss[0:64, :], AF.Relu)

                    ps2 = ep2.tile([128, PC], F32, tag="ps2")
                    nc.tensor.matmul(
                        ps2[:], lhsT=w["w2t"][:], rhs=h1e[:], start=True, stop=True
                    )
                    h2e = esb.tile([128, PC], F16, tag="h2e")
                    nc.scalar.activation(h2e[:], ps2[:], AF.Relu, bias=w["b2c"][:])

                    po = epo.tile([128, PC], F32, tag="po")
                    for j in range(PC // 128):
                        nc.tensor.matmul(
                            po[:, j * 128 : (j + 1) * 128],
                            lhsT=h2e[:, j * 128 : (j + 1) * 128],
                            rhs=w["w3t"][:],
                            start=True,
                            stop=True,
                        )
                    oute = esb.tile([128, PC], F16, tag="oute")
                    nc.vector.tensor_add(oute[:], po[:], w["b3b"][:])

                    src = oute[:].rearrange("p (g o) -> p g o", g=4)
                    if s == "f":
                        if c < NFR // PC:
                            nc.sync.dma_start(out=fview[c, :, :, :], in_=src)
                        else:
                            nc.sync.dma_start(
                                out=gview[c - NFR // PC, :, :, :], in_=src
                            )
                    else:
                        if c < NPH // PC:
                            nc.sync.dma_start(out=pview[c, :, :, :], in_=src)
                        else:
                            nc.sync.dma_start(
                                out=aview[:, :, :],
                                in_=oute[0:2, :].rearrange("p (j o) -> p j o", j=4),
                            )

                pe_jobs = []
                if POSENC_ON:
                    pe_jobs = [("f", idxf_t, c) for c in range(NFC)] + [
                        ("a", idxa_t, c) for c in range(NAC)
                    ]
                n_pairs = NDC // 2 if DIST_ON else 0
                emitted = 0
                # posenc has a much shorter input lead-in than dist (which
                # waits on the log1p -> DRAM scratch round-trip); start with a
                # few posenc chunks so the PE isn't idle at kernel start
                for _ in range(min(5, len(pe_jobs))):
                    posenc_chunk(*pe_jobs[emitted])
                    emitted += 1
                for t in range(n_pairs):
                    ot = dsb.tile([128, 2 * PC], F16, tag="ot")
                    ct = dsb.tile([1, 2 * PC], F16, tag="ct")
                    nc.gpsimd.dma_start(out=ct[:], in_=scr[2 * t : 2 * t + 2, :])
                    for h in range(2):
                        dist_chunk(
                            2 * t + h,
                            ct[0:1, h * PC : (h + 1) * PC],
                            ot[:, h * PC : (h + 1) * PC],
                        )
                    nc.sync.dma_start(
                        out=dview[t, :, :, :, :],
                        in_=ot[:].rearrange("p (h g o) -> p h g o", h=2, g=4),
                    )
                    want = min(len(pe_jobs), (t + 1) * len(pe_jobs) // max(1, n_pairs - 8))
                    while emitted < want:
                        posenc_chunk(*pe_jobs[emitted])
                        emitted += 1
                while emitted < len(pe_jobs):
                    posenc_chunk(*pe_jobs[emitted])
                    emitted += 1
    return nc


def _get_program():
    global _PROG
    if _PROG is None:
        _PROG = _build_program()
    return _PROG


def _perm512(a):
    """Permute rows within each 512 block: new position j*128+p <- old 4p+j."""
    n = a.shape[0]
    assert n % 512 == 0
    rest = a.shape[1:]
    return np.ascontiguousarray(
        a.reshape(n // 512, 128, 4, *rest).swapaxes(1, 2).reshape(a.shape)
    )


def _host_inputs(inputs):
    """Build the per-core in_maps from the full problem inputs."""
    g = {k: np.asarray(v) for k, v in inputs.items()}
    sz = np.array([float(g["sz_r"]), float(g["sz_c"])], dtype=np.float64)

    def f32(x):
        return np.ascontiguousarray(np.asarray(x, dtype=np.float32))

    def f16(x):
        return np.ascontiguousarray(np.asarray(x, dtype=np.float16))

    # args columns of the per-branch scaled matmul (cols 64..103): fp16 is
    # exact for the 2^k/sz coefficients and integer idx; only the pi/2 shift
    # rounds (~5e-4 rad).
    Margs = np.zeros((3, 40), dtype=np.float64)
    for j in range(40):
        blk = j if j < 20 else j - 20
        k, cc = blk // 2, blk % 2
        Margs[cc, j] = (2.0**k) / sz[cc]
        Margs[2, j] = 0.0 if j < 20 else np.pi / 2

    Wd1, bd1 = g["Wd1"], g["bd1"]
    Wd2, bd2 = g["Wd2"], g["bd2"]
    consts = {
        "w1d": f32(Wd1[:, 0:1]),
        "b1d": f32(bd1.reshape(128, 1)),
        "w2td": f16(np.asarray(Wd2, np.float64).T),
        "b2bd": f32(np.tile(bd2.reshape(1, -1), (128, PC // 128))),
    }

    # posenc weights; X row order -> original pe column order
    sin_src = [2 + 4 * (i // 2) + (i % 2) for i in range(20)]
    cos_src = [2 + 4 * (i // 2) + 2 + (i % 2) for i in range(20)]
    cols = sin_src + cos_src
    for s, W1, b1, W2, b2, W3, b3, tv in (
        ("f", g["Wf1"], g["bf1"], g["Wf2"], g["bf2"], g["Wf3"], g["bf3"], (1.0, 0.0)),
        ("a", g["Wa1"], g["ba1"], g["Wa2"], g["ba2"], g["Wa3"], g["ba3"], (0.0, 1.0)),
    ):
        W1 = np.asarray(W1, np.float64)
        b1e = np.asarray(b1, np.float64) + W1[:, 42] * tv[0] + W1[:, 43] * tv[1]
        consts[f"w1a{s}"] = f16(W1[:, cols].T)               # [40, 64]
        Mb = np.zeros((3, 128), dtype=np.float64)
        Mb[0:2, 0:64] = (W1[:, 0:2] / sz[None, :]).T           # coords partial
        Mb[2, 0:64] = b1e                                      # bias via ones row
        Mb[:, 64:104] = Margs
        consts[f"mm{s}"] = f16(Mb)
        consts[f"w2t{s}"] = f16(np.asarray(W2, np.float64).T)  # [64, 128]
        consts[f"b2c{s}"] = f32(np.asarray(b2).reshape(128, 1))
        consts[f"w3t{s}"] = f16(np.asarray(W3, np.float64).T)  # [128, 128]
        consts[f"b3b{s}"] = f32(np.tile(np.asarray(b3).reshape(1, -1), (128, PC // 128)))

    in_maps = []
    fi, gi = g["frontier_idx"], g["ghistory_idx"]
    pi_, ai = g["phistory_idx"], g["agent_pos"]
    dv = g["dist_vals"]
    for b in range(B):
        idxf = np.concatenate([np.asarray(fi[b]), np.asarray(gi[b])], axis=0)
        idxa_raw = np.concatenate([np.asarray(pi_[b]), np.asarray(ai[b])], axis=0)
        idxa = np.zeros((NAB, 2), dtype=np.float64)
        idxa[: idxa_raw.shape[0]] = np.asarray(idxa_raw, np.float64)

        idxf_p = _perm512(np.asarray(idxf, np.float64))
        idxa_p = _perm512(idxa)

        def with_ones(t, n):
            out = np.ones((3, n), dtype=np.float16)
            out[0:2, :] = t.T.astype(np.float16)
            return np.ascontiguousarray(out)

        m = dict(consts)
        m["idxf"] = with_ones(idxf_p, NFB)
        m["idxa"] = with_ones(idxa_p, NAB)
        dperm = _perm512(np.asarray(dv[b], np.float32).reshape(ND))
        m["dist"] = f32(dperm.reshape(128, 512))
        in_maps.append(m)
    return in_maps


def kernel(**inputs):
    global LAST_RESULT, _LEGALIZED
    nc = _get_program()
    if not _LEGALIZED:
        # CoreSim can't execute the injected EventSemaphores, so this runs
        # only on the hardware path.
        _legalize_waits(nc)
        _LEGALIZED = True
    in_maps = _host_inputs(inputs)
    trace = os.environ.get("BASS_TRACE", "") not in ("", "0")
    if trace:
        try:
            from antenv.axon_hooks import get_axon_ntff_profile_hook  # noqa: F401
        except ImportError:
            # profiling hook unavailable; run without trace (the env var is
            # re-read inside run_bass_kernel_spmd, so pin it off too)
            trace = False
            os.environ["BASS_NEVER_TRACE"] = "1"
    res = run_bass_kernel_spmd(nc, in_maps, core_ids=list(range(B)), trace=trace)
    LAST_RESULT = res
    r = res.results
    dist_enc = np.stack([r[b]["dist_out"] for b in range(B)]).astype(np.float32)
    frontier = np.stack([r[b]["front_out"] for b in range(B)]).astype(np.float32)
    agent = np.stack([r[b]["agent_out"] for b in range(B)]).astype(np.float32)
    phist = np.stack([r[b]["phist_out"] for b in range(B)]).astype(np.float32)
    ghist = np.stack([r[b]["ghist_out"] for b in range(B)]).astype(np.float32)
    return (dist_enc, frontier, agent, phist, ghist)
